# revision 1
# baseline (speedup 1.0000x reference)
"""Bass/Trainium2 kernel for nn_BuildLstmUnrollNet.

Problem: 2-layer LSTM, unrolled T=11 steps with per-step (non-shared)
weights, B=8192, R=425, IN=20.  Output block t is the last-layer h
*before* step t, so only steps 0..9 need computing (step 10's weights
never affect the output).

Strategy (data-parallel over batch, 8 cores x 1024 rows):
  - States kept batch-major in ONE packed bf16 buffer per m-tile:
    cols [h0(425) | 1.0 | x(20) | h1(425) | pad(25)] = 896 = 7*128.
    Gates are computed batch-major in PSUM with the *transposed
    activations* stationary (lhsT) and the weights as the moving
    operand: layer 0 contracts over packed rows 0..511 (4 K-passes,
    bias + x folded in for free), layer 1 over rows 0..895 (7 K-passes
    -- h1 rides in the same packed buffer, so no ceil() waste).
  - Weights are host-prepacked+transposed to [K, 4R] bf16 blocks whose
    row layout matches the packed state buffer exactly.
  - The recurrent transpose h -> hT bounces through DRAM so the x-bar
    DMA transpose can do few, large [rows,128] -> [128,rows] blocks on
    the SP/HWDGE queue (no compute-engine cycles); the h0' chunks are
    transposed in 3 row-groups as the layer-0 cells complete so layer
    1's first batch tiles unblock early.
  - Cell math: ACT (one fused sigmoid over i|f|o + tanh straight out of
    PSUM), DVE muls/adds; c0/c1 stay fp32; h1 output written fp32.

kernel(**inputs) takes full-size numpy inputs, does the host-side
packing/sharding, runs the same program SPMD on cores 0..7, and
reassembles the full [8192, 4675] fp32 output (block 0 comes straight
from the initial state on the host).
"""

import numpy as np
import ml_dtypes

BF16 = ml_dtypes.bfloat16

B = 8192
NCORES = 8
BC = B // NCORES          # batch rows per core (1024)
NB = BC // 128            # m-tiles per core (8)
R = 425
IN = 20
GN = 4 * R                # 1700 gate columns
H1OFF = R + 1 + IN        # 446: h1 col offset in the packed state block
HC = 896                  # packed state block width (7*128)
NKC = HC // 128           # 7 transpose chunks
NK0 = 4                   # layer-0 K-passes (rows 0..511)
NK1 = 7                   # layer-1 K-passes (rows 0..895)
NKT = NK0 + NK1           # 11 weight K-blocks per step
NSTEPS = 10
# N chunks of the 1700-wide gate output (one PSUM bank each)
NCHUNKS = [(0, 512), (512, 512), (1024, 512), (1536, 164)]

# set by test.py to profile; results stashed in LAST_RESULT
TRACE = False
LAST_RESULT = None


def build_bass(n_steps=NSTEPS, finalize=True):
    import concourse.bacc as bacc
    import concourse.mybir as mybir
    import concourse.tile as tile

    f32 = mybir.dt.float32
    bf16 = mybir.dt.bfloat16
    Sig = mybir.ActivationFunctionType.Sigmoid
    Tanh = mybir.ActivationFunctionType.Tanh

    nc = bacc.Bacc()

    w_d = nc.declare_dram_parameter("w", [n_steps, 128, NKT * GN], bf16, False)
    hci_d = nc.declare_dram_parameter("hci", [128, NB * HC], bf16, False)
    htci_d = nc.declare_dram_parameter("htci", [128, NKC * BC], bf16, False)
    c0i_d = nc.declare_dram_parameter("c0i", [128, NB * R], f32, False)
    c1i_d = nc.declare_dram_parameter("c1i", [128, NB * R], f32, False)
    out_d = nc.declare_dram_parameter("out", [BC, n_steps * R], f32, True)
    # DRAM bounce buffer for the recurrent transpose (batch-major packed h)
    hd = nc.dram_tensor("hd", [BC, HC], bf16)

    with tile.TileContext(nc) as tc:
        with (
            tc.tile_pool(name="consts", bufs=1) as consts,
            tc.tile_pool(name="wpool", bufs=2) as wpool,
            tc.tile_pool(name="gpsum", bufs=2, space="PSUM") as gpsum,
            tc.tile_pool(name="tmp", bufs=3) as tmp,
        ):
            # persistent state tiles
            hs_t = consts.tile([128, NB * HC], bf16)   # packed batch-major
            htc = consts.tile([128, NKC * BC], bf16)   # transposed (lhsT)
            c0 = consts.tile([128, NB * R], f32)
            c1 = consts.tile([128, NB * R], f32)
            h1f = consts.tile([128, NB * R], f32)      # fp32 h1 for output

            # init DMAs on the SP (HWDGE) queue, most-urgent first, while
            # the first weight chunks stream on the Pool (SWDGE) queue
            for k in range(NKC):
                nc.sync.dma_start(htc[:, k * BC: (k + 1) * BC],
                                  htci_d[:, k * BC: (k + 1) * BC])
            nc.sync.dma_start(c0[:], c0i_d[:])
            nc.sync.dma_start(hs_t[:], hci_d[:])
            nc.sync.dma_start(c1[:], c1i_d[:])

            # step-0 weights, split per k-block so matmuls start early
            w = wpool.tile([128, NKT * GN], bf16, tag="w")
            for k in range(NKT):
                nc.gpsimd.dma_start(w[:, k * GN: (k + 1) * GN],
                                    w_d[0][:, k * GN: (k + 1) * GN])

            # PE warm-up: the HAM clock gate needs ~3.4us of sustained
            # activity before the PE runs at full rate.  Burn the initial
            # DMA wait with dummy matmuls on zeroed scratch so the ramp
            # clock starts before the real work does.
            warm = consts.tile([128, 128], bf16)
            nc.vector.memset(warm[:], 0.0)
            wps = gpsum.tile([128, 512], f32, tag="g")
            for i in range(20):
                nc.tensor.matmul(wps[:, 0: 128], warm[:], warm[:],
                                 start=True, stop=True)

            for t in range(n_steps):
                if t < n_steps - 1:
                    # next step's weights: few bulk chunks on the Pool queue
                    w_next = wpool.tile([128, NKT * GN], bf16, tag="w")
                    for c in range(4):
                        lo = c * 3 * GN
                        hi = min((c + 1) * 3 * GN, NKT * GN)
                        nc.gpsimd.dma_start(
                            w_next[:, lo: hi], w_d[t + 1][:, lo: hi])

                if t > 0:
                    # refresh the h1 rows (chunks 4..6) of the transposed
                    # state: h1^{t} was bounced to DRAM at the end of step
                    # t-1; layer 1 of this step reads it
                    for half in range(2):
                        rows = slice(half * 512, (half + 1) * 512)
                        for k in range(NK0, NKC):
                            nc.sync.dma_start(
                                out=htc[:, k * BC + half * 512:
                                        k * BC + (half + 1) * 512],
                                in_=hd[rows, k * 128: (k + 1) * 128],
                                transpose=True)

                for layer in range(2):
                    if layer == 0:
                        # (k-chunk of htc, W k-block)
                        kplan = [(k, k) for k in range(NK0)]
                    else:
                        # h1-only chunks (4..6) first: they are ready from
                        # the top-of-step transposes; the h0' chunks (0..3)
                        # are transposed mid-step after the layer-0 cells
                        kplan = ([(k, NK0 + k) for k in range(NK0, NKC)]
                                 + [(k, NK0 + k) for k in range(NK0)])
                    nk = len(kplan)
                    cst = c0 if layer == 0 else c1
                    for m in range(NB):
                        g = gpsum.tile([128, GN], f32, tag="g")
                        for ki, (kk, wk) in enumerate(kplan):
                            lhsT = htc[:, kk * BC + m * 128:
                                       kk * BC + (m + 1) * 128]
                            for (no, nw) in NCHUNKS:
                                nc.tensor.matmul(
                                    g[:, no: no + nw],
                                    lhsT,
                                    w[:, wk * GN + no: wk * GN + no + nw],
                                    start=(ki == 0),
                                    stop=(ki == nk - 1),
                                )

                        # LSTM cell (torch gate order: i, f, o, g).  One
                        # sigmoid over the contiguous i|f|o columns, one tanh.
                        cs = cst[:, m * R: (m + 1) * R]
                        tsig = tmp.tile([128, 3 * R], f32, tag="tsig")
                        nc.scalar.activation(tsig[:], g[:, 0: 3 * R], Sig)
                        ti = tsig[:, 0: R]
                        tf = tsig[:, R: 2 * R]
                        to = tsig[:, 2 * R: 3 * R]
                        tg = tmp.tile([128, R], f32, tag="tg")
                        nc.scalar.activation(tg[:], g[:, 3 * R: 4 * R], Tanh)

                        tig = tmp.tile([128, R], f32, tag="tig")
                        nc.vector.tensor_mul(tig[:], ti, tg[:])
                        tfc = tmp.tile([128, R], f32, tag="tfc")
                        nc.vector.tensor_mul(tfc[:], tf, cs)
                        nc.vector.tensor_add(cs, tfc[:], tig[:])
                        ttc = tmp.tile([128, R], f32, tag="ttc")
                        nc.scalar.activation(ttc[:], cs, Tanh)

                        # h writes + transposes ride the SP/HWDGE queue
                        # (cheap per-op); bulk W + out stores ride Pool/SWDGE
                        if layer == 0:
                            # h0_new -> packed bf16 cols 0..424, bounce the
                            # first 512 cols (incl ones/x consts and the
                            # still-current h1 rows 0..65) to DRAM
                            nc.vector.tensor_mul(
                                hs_t[:, m * HC: m * HC + R], to, ttc[:])
                            nc.sync.dma_start(
                                hd[m * 128: (m + 1) * 128, 0: 512],
                                hs_t[:, m * HC: m * HC + 512])
                        else:
                            hh = h1f[:, m * R: (m + 1) * R]
                            nc.vector.tensor_mul(hh, to, ttc[:])
                            nc.gpsimd.dma_start(
                                out_d[m * 128: (m + 1) * 128,
                                      t * R: (t + 1) * R], hh)
                            if t < n_steps - 1:
                                nc.vector.tensor_copy(
                                    hs_t[:, m * HC + H1OFF:
                                         m * HC + H1OFF + R], hh)
                                nc.sync.dma_start(
                                    hd[m * 128: (m + 1) * 128, 512: HC],
                                    hs_t[:, m * HC + 512: (m + 1) * HC])

                        # mid-step transpose of h0' chunk rows as soon as
                        # their m-tiles are written (3-way split: after m2,
                        # m5, m7) so layer 1's first M-tiles unblock early
                        if layer == 0 and m in (2, 5, 7):
                            lo = {2: 0, 5: 384, 7: 768}[m]
                            hi = {2: 384, 5: 768, 7: 1024}[m]
                            for k in range(NK0):
                                nc.sync.dma_start(
                                    out=htc[:, k * BC + lo: k * BC + hi],
                                    in_=hd[lo: hi, k * 128: (k + 1) * 128],
                                    transpose=True)
                if t < n_steps - 1:
                    w = w_next
    if finalize:
        nc.finalize()
    return nc


def _pack_pf(a):
    """[BC, C] -> [128, NB*C] with m-tile m at cols m*C."""
    c = a.shape[1]
    return np.ascontiguousarray(
        a.reshape(NB, 128, c).transpose(1, 0, 2).reshape(128, NB * c))


def _pack_kt(a):
    """[BC, HC] -> transposed [128, NKC*BC] with K-chunk k at cols k*BC."""
    return np.ascontiguousarray(
        a.T.reshape(NKC, 128, BC).transpose(1, 0, 2).reshape(128, NKC * BC))


def prep_inputs(x, init_states_input, W_i2h0, b_i2h0, W_h2h0, b_h2h0,
                W_i2h1, b_i2h1, W_h2h1, b_h2h1, n_steps=NSTEPS):
    """Host-side packing.  Returns (in_maps, h1_init_full)."""
    x = np.asarray(x, np.float32)
    init = np.asarray(init_states_input, np.float32)
    W_i2h0 = np.asarray(W_i2h0, np.float32)
    b_i2h0 = np.asarray(b_i2h0, np.float32)
    W_h2h0 = np.asarray(W_h2h0, np.float32)
    b_h2h0 = np.asarray(b_h2h0, np.float32)
    W_i2h1 = np.asarray(W_i2h1, np.float32)
    b_i2h1 = np.asarray(b_i2h1, np.float32)
    W_h2h1 = np.asarray(W_h2h1, np.float32)
    b_h2h1 = np.asarray(b_h2h1, np.float32)

    # per-step weight blocks, K-major, transposed to [K, 4R], rows
    # matching the packed state layout [h0 | 1 | x | h1 | pad]
    Wd = np.zeros((n_steps, NKT * 128, GN), np.float32)
    for t in range(n_steps):
        # layer-0 K-rows 0..511
        Wd[t, 0:R] = W_h2h0[t].T
        Wd[t, R] = b_i2h0[t] + b_h2h0[t]
        Wd[t, R + 1: R + 1 + IN] = W_i2h0[t].T
        # layer-1 K-rows 0..895 at block offset 4*128=512
        o = NK0 * 128
        Wd[t, o: o + R] = W_i2h1[t].T
        Wd[t, o + R] = b_i2h1[t] + b_h2h1[t]
        Wd[t, o + H1OFF: o + H1OFF + R] = W_h2h1[t].T
    w_dev = np.ascontiguousarray(
        Wd.reshape(n_steps, NKT, 128, GN).transpose(0, 2, 1, 3)
        .reshape(n_steps, 128, NKT * GN)).astype(BF16)

    init4 = init.reshape(B, 4, R)
    h0_full, c0_full = init4[:, 0], init4[:, 1]
    h1_full, c1_full = init4[:, 2], init4[:, 3]

    in_maps = []
    for c in range(NCORES):
        sl = slice(c * BC, (c + 1) * BC)
        hcp = np.zeros((BC, HC), np.float32)
        hcp[:, 0:R] = h0_full[sl]
        hcp[:, R] = 1.0
        hcp[:, R + 1: R + 1 + IN] = x[sl]
        hcp[:, H1OFF: H1OFF + R] = h1_full[sl]
        hcp = hcp.astype(BF16)
        in_maps.append({
            "w": w_dev,
            "hci": _pack_pf(hcp),
            "htci": _pack_kt(hcp),
            "c0i": _pack_pf(np.ascontiguousarray(c0_full[sl])),
            "c1i": _pack_pf(np.ascontiguousarray(c1_full[sl])),
        })
    return in_maps, h1_full


def kernel(x, init_states_input, W_i2h0, b_i2h0, W_h2h0, b_h2h0,
           W_i2h1, b_i2h1, W_h2h1, b_h2h1):
    global LAST_RESULT
    from concourse.bass_utils import run_bass_kernel_spmd

    in_maps, h1_full = prep_inputs(
        x, init_states_input, W_i2h0, b_i2h0, W_h2h0, b_h2h0,
        W_i2h1, b_i2h1, W_h2h1, b_h2h1)

    nc = build_bass(NSTEPS)
    res = run_bass_kernel_spmd(nc, in_maps, list(range(NCORES)), trace=TRACE)
    LAST_RESULT = res

    out = np.empty((B, (NSTEPS + 1) * R), np.float32)
    out[:, 0:R] = h1_full
    for c in range(NCORES):
        out[c * BC: (c + 1) * BC, R:] = res.results[c]["out"]
    return out



# revision 8
# speedup vs baseline: 1.0348x; 1.0348x over previous
"""Bass/Trainium2 kernel for nn_BuildLstmUnrollNet (bf16+fp8 hybrid).

Problem: 2-layer LSTM, unrolled T=11 steps with per-step (non-shared)
weights, B=8192, R=425, IN=20.  Output block t is the last-layer h
*before* step t, so only steps 0..9 need computing.

Strategy (data-parallel over batch, 8 cores x 1024 rows):
  - Step 0 runs its matmuls in bf16 (the initial h/c are raw unbounded
    randn; fp8-quantizing them costs ~6x the tolerable error).  Steps
    1..9 run all matmuls in fp8e4 with perf_mode=DoubleRow (2 K-rows
    per PE cell, 2x throughput): weights are the moving operand
    (pre-scaled x256, g-gate columns additionally x2), transposed
    activations are stationary, 256 K-features per pass.  Post-step-0
    h's are tanh-bounded, so fp8 keeps absmax rel err ~4x under the
    2e-2 gate.
  - Packed batch-major state (bf16): [h0(425) | 1 | x(20) | pad ->512 |
    h1(425) | pad ->1024].  Bias rides the ones column; layer 0
    contracts features 0..511, layer 1 contracts 0..1023 (x rows
    zero-weighted).  The recurrent transpose bounces through DRAM in
    bf16 (2-byte x-bar DMA transpose) into a [128, 8 chunks, 1024]
    K-major buffer; the otherwise-idle GPSIMD/Pool engine then copies
    it to fp8.  DoubleRow pairs adjacent 128-row chunks (pair stride
    1024, 16-aligned, per the s3_lw dual-fp8 ISA restrictions).
  - One single Sigmoid ACT op per (m-tile, layer) covers ALL 1700 gate
    columns: tanh(g) = 2*sigmoid(2g)-1 with the g columns' weights
    doubled on the host; the affine fix-up is a cheap 4x-mode DVE
    tensor_scalar.  PSUM descale (1/256) rides the ACT scale input.
  - Cell math in fp16 on DVE (2x mode), batched over 4-m-tile groups;
    tanh(c) on ACT batched per group.
  - h1 output is stored bf16 (straight from the packed state) and
    upcast on the host.

kernel(**inputs) takes full-size numpy inputs, does the host-side
packing/sharding, runs the same program SPMD on cores 0..7, and
reassembles the full [8192, 4675] fp32 output (block 0 comes straight
from the initial state on the host).
"""

import numpy as np
import ml_dtypes

FP8 = ml_dtypes.float8_e4m3     # TRN float8e4: max normal 240, inf above
BF16 = ml_dtypes.bfloat16
FP16 = np.float16

B = 8192
NCORES = 8
BC = B // NCORES          # batch rows per core (1024)
NB = BC // 128            # m-tiles per core (8)
R = 425
IN = 20
GN = 4 * R                # 1700 gate columns
GNP = 1712                # padded gate cols in the fp8 weight tile (16|GNP)
SW = 256.0                # fp8 weight scale, descaled via ACT scale=1/SW
ONES_COL = R              # 425: ones feature (bias row rides here)
X_COL = R + 1             # 426..445: x features
H1_OFF = 512              # h1 features at 512..936
SB = 1024                 # packed state width = 8 chunks of 128
NSLOT = 6                 # fp8 weight pair-slots: L0 p0,p1 + L1 p0..p3
NCHUNKS = [(0, 512), (512, 512), (1024, 512), (1536, 164)]
GRPS = [(0, 4), (4, 4)]   # m-tile groups for the cell-math pipeline
NSTEPS = 10

# set by test.py to profile; results stashed in LAST_RESULT
TRACE = False
LAST_RESULT = None


def build_bass(n_steps=NSTEPS, finalize=True):
    import concourse.bacc as bacc
    import concourse.mybir as mybir
    import concourse.tile as tile

    f32 = mybir.dt.float32
    bf16 = mybir.dt.bfloat16
    fp16 = mybir.dt.float16
    f8 = mybir.dt.float8e4
    Sig = mybir.ActivationFunctionType.Sigmoid
    Tanh = mybir.ActivationFunctionType.Tanh
    DR = mybir.MatmulPerfMode.DoubleRow
    mult = mybir.AluOpType.mult
    add = mybir.AluOpType.add

    nc = bacc.Bacc()

    w_d = nc.declare_dram_parameter("w", [n_steps, 128, NSLOT, 2, GNP], f8,
                                    False)
    wb_d = nc.declare_dram_parameter("wb", [128, 12, GN], bf16, False)
    htci_d = nc.declare_dram_parameter("htci", [128, 8, BC], bf16, False)
    hsbi_d = nc.declare_dram_parameter("hsbi", [128, NB, SB], bf16, False)
    c0i_d = nc.declare_dram_parameter("c0i", [128, NB, R], fp16, False)
    c1i_d = nc.declare_dram_parameter("c1i", [128, NB, R], fp16, False)
    out_d = nc.declare_dram_parameter("out", [BC, n_steps * R], bf16, True)
    # DRAM bounce buffer for the recurrent transposes
    hd = nc.dram_tensor("hd", [BC, SB], bf16)

    # fp8 pair-slot -> first transposed-state chunk (pairs (c, c+1))
    SLOT_CH = {0: 0, 1: 2, 2: 0, 3: 2, 4: 4, 5: 6}
    L_SLOTS = [(0, 1), (2, 3, 4, 5)]
    L_NCH = [4, 8]            # step-0 bf16 K-chunks per layer

    with tile.TileContext(nc) as tc:
        with (
            tc.tile_pool(name="consts", bufs=1) as consts,
            tc.tile_pool(name="wpool", bufs=2) as wpool,
            tc.tile_pool(name="w0pool", bufs=1) as w0pool,
            tc.tile_pool(name="gpsum", bufs=2, space="PSUM") as gpsum,
        ):
            # persistent state tiles
            hsb = consts.tile([128, NB, SB], bf16)      # packed batch-major
            htcb = consts.tile([128, 8, BC], bf16)      # transposed bf16
            htcu8 = consts.tile([128, 8, BC], f8)       # transposed fp8
            c0 = consts.tile([128, NB, R], fp16)
            c1 = consts.tile([128, NB, R], fp16)
            ts = consts.tile([128, NB, GN], fp16)       # sigmoid outputs
            tg16 = consts.tile([128, NB, R], fp16)
            tc16 = consts.tile([128, NB, R], fp16)
            tig = consts.tile([128, NB, R], fp16)
            tfc = consts.tile([128, NB, R], fp16)

            # step-0 bf16 weights (w0pool, dead after step 0)
            wb0 = w0pool.tile([128, 4, GN], bf16)
            wb1 = w0pool.tile([128, 8, GN], bf16)

            # init DMAs on the SP (HWDGE) queue, most-urgent first; step-0
            # bf16 weights stream on the Pool (SWDGE) queue
            for c in range(4):
                nc.sync.dma_start(htcb[:, c], htci_d[:, c])
            nc.gpsimd.dma_start(wb0[:], wb_d[:, 0:4])
            nc.sync.dma_start(c0[:], c0i_d[:])
            nc.sync.dma_start(hsb[:], hsbi_d[:])
            for c in range(4, 8):
                nc.sync.dma_start(htcb[:, c], htci_d[:, c])
            nc.sync.dma_start(c1[:], c1i_d[:])
            for q in range(4):
                nc.gpsimd.dma_start(wb1[:, 2 * q:2 * q + 2],
                                    wb_d[:, 4 + 2 * q:6 + 2 * q])

            # PE warm-up: the HAM clock gate needs ~3.4us of sustained
            # activity before the PE runs at full rate.
            warm = consts.tile([128, 128], bf16)
            nc.vector.memset(warm[:], 0.0)
            wps = gpsum.tile([128, 512], f32, tag="g")
            for i in range(20):
                nc.tensor.matmul(wps[:, 0:128], warm[:], warm[:],
                                 start=True, stop=True)

            w = None
            for t in range(n_steps):
                if t < n_steps - 1:
                    # next step's fp8 weights on the Pool queue
                    w_next = wpool.tile([128, NSLOT, 2, GNP], f8, tag="w")
                    for q in range(3):
                        nc.gpsimd.dma_start(w_next[:, 2 * q:2 * q + 2],
                                            w_d[t + 1][:, 2 * q:2 * q + 2])

                for layer in range(2):
                    cst = c0 if layer == 0 else c1
                    for (g0, gl) in GRPS:
                        for m in range(g0, g0 + gl):
                            g = gpsum.tile([128, GN], f32, tag="g")
                            if t == 0:
                                wbt = wb0 if layer == 0 else wb1
                                nch = L_NCH[layer]
                                for ki in range(nch):
                                    lhsT = htcb[:, ki, m * 128:(m + 1) * 128]
                                    for (no, nw) in NCHUNKS:
                                        nc.tensor.matmul(
                                            g[:, no:no + nw],
                                            lhsT,
                                            wbt[:, ki, no:no + nw],
                                            start=(ki == 0),
                                            stop=(ki == nch - 1),
                                        )
                            else:
                                slots = L_SLOTS[layer]
                                for si, s in enumerate(slots):
                                    ch = SLOT_CH[s]
                                    lhsT = htcu8[:, ch:ch + 2,
                                                 m * 128:(m + 1) * 128]
                                    for (no, nw) in NCHUNKS:
                                        nc.tensor.matmul(
                                            g[:, no:no + nw],
                                            lhsT,
                                            w[:, s, :, no:no + nw],
                                            start=(si == 0),
                                            stop=(si == len(slots) - 1),
                                            perf_mode=DR,
                                        )
                            # one sigmoid covers all gates:
                            # tanh(g) = 2*sigmoid(2g)-1, g-col weights x2
                            nc.scalar.activation(
                                ts[:, m], g[:], Sig,
                                scale=(1.0 if t == 0 else 1.0 / SW))

                        # batched fp16 cell math for the group
                        ms = slice(g0, g0 + gl)
                        s_i = ts[:, ms, 0:R]
                        s_f = ts[:, ms, R:2 * R]
                        s_o = ts[:, ms, 2 * R:3 * R]
                        s_g = ts[:, ms, 3 * R:4 * R]
                        nc.vector.tensor_scalar(tg16[:, ms], s_g, 2.0, -1.0,
                                                mult, add)
                        nc.vector.tensor_mul(tig[:, ms], s_i, tg16[:, ms])
                        nc.vector.tensor_mul(tfc[:, ms], s_f, cst[:, ms])
                        nc.vector.tensor_add(cst[:, ms], tfc[:, ms],
                                             tig[:, ms])
                        nc.scalar.activation(tc16[:, ms], cst[:, ms], Tanh)

                        rows = slice(g0 * 128, (g0 + gl) * 128)
                        hoff = 0 if layer == 0 else H1_OFF
                        # h write into the packed bf16 state
                        nc.vector.tensor_mul(hsb[:, ms, hoff:hoff + R], s_o,
                                             tc16[:, ms])
                        if layer == 1:
                            for m in range(g0, g0 + gl):
                                nc.gpsimd.dma_start(
                                    out_d[m * 128:(m + 1) * 128,
                                          t * R:(t + 1) * R],
                                    hsb[:, m, H1_OFF:H1_OFF + R])
                        if layer == 0 or t < n_steps - 1:
                            # bounce the updated packed half to DRAM, x-bar
                            # transpose back K-major (bf16), then fp8-ify on
                            # the Pool engine for the DoubleRow stationary
                            for m in range(g0, g0 + gl):
                                nc.sync.dma_start(
                                    hd[m * 128:(m + 1) * 128,
                                       hoff:hoff + 512],
                                    hsb[:, m, hoff:hoff + 512])
                            ch0 = 0 if layer == 0 else 4
                            for ch in range(ch0, ch0 + 4):
                                nc.sync.dma_start(
                                    out=htcb[:, ch, rows],
                                    in_=hd[rows, 128 * ch:128 * ch + 128],
                                    transpose=True)
                            nc.gpsimd.tensor_copy(
                                htcu8[:, ch0:ch0 + 4, rows],
                                htcb[:, ch0:ch0 + 4, rows])
                if t < n_steps - 1:
                    w = w_next
    if finalize:
        nc.finalize()
    return nc


def prep_inputs(x, init_states_input, W_i2h0, b_i2h0, W_h2h0, b_h2h0,
                W_i2h1, b_i2h1, W_h2h1, b_h2h1, n_steps=NSTEPS):
    """Host-side packing.  Returns (in_maps, h1_init_full)."""
    x = np.asarray(x, np.float32)
    init = np.asarray(init_states_input, np.float32)
    W_i2h0 = np.asarray(W_i2h0, np.float32)[:n_steps]
    b_i2h0 = np.asarray(b_i2h0, np.float32)[:n_steps]
    W_h2h0 = np.asarray(W_h2h0, np.float32)[:n_steps]
    b_h2h0 = np.asarray(b_h2h0, np.float32)[:n_steps]
    W_i2h1 = np.asarray(W_i2h1, np.float32)[:n_steps]
    b_i2h1 = np.asarray(b_i2h1, np.float32)[:n_steps]
    W_h2h1 = np.asarray(W_h2h1, np.float32)[:n_steps]
    b_h2h1 = np.asarray(b_h2h1, np.float32)[:n_steps]

    # per-step K-major weight blocks, rows matching the packed state
    WL0 = np.zeros((n_steps, 512, GN), np.float32)
    WL0[:, 0:R] = W_h2h0.transpose(0, 2, 1)
    WL0[:, ONES_COL] = b_i2h0 + b_h2h0
    WL0[:, X_COL:X_COL + IN] = W_i2h0.transpose(0, 2, 1)
    WL1 = np.zeros((n_steps, SB, GN), np.float32)
    WL1[:, 0:R] = W_i2h1.transpose(0, 2, 1)
    WL1[:, ONES_COL] = b_i2h1 + b_h2h1
    WL1[:, H1_OFF:H1_OFF + R] = W_h2h1.transpose(0, 2, 1)
    for Wx in (WL0, WL1):
        Wx[:, :, 3 * R:] *= 2.0     # g-cols doubled: tanh via sigmoid

    # step-0 bf16 weights: 12 K-chunks of 128 (L0 c0..3, L1 c0..7)
    wb = np.concatenate([WL0[0].reshape(4, 128, GN),
                         WL1[0].reshape(8, 128, GN)], axis=0) \
        .transpose(1, 0, 2)                       # [128, 12, GN]
    wb = np.ascontiguousarray(wb.astype(BF16))

    # fp8 step weights: pair-slot j covers chunks (2j, 2j+1);
    # k = 128*(2j+i) + p  ->  [T, p, slot, i, n], n padded to GNP
    w8f = np.concatenate([
        WL0.reshape(n_steps, 2, 2, 128, GN).transpose(0, 3, 1, 2, 4),
        WL1.reshape(n_steps, 4, 2, 128, GN).transpose(0, 3, 1, 2, 4),
    ], axis=2) * SW                               # [T, 128, 6, 2, GN]
    w8 = np.zeros((n_steps, 128, NSLOT, 2, GNP), FP8)
    w8[..., :GN] = FP8(np.clip(w8f, -240.0, 240.0))

    init4 = init.reshape(B, 4, R)
    h0_full, c0_full = init4[:, 0], init4[:, 1]
    h1_full, c1_full = init4[:, 2], init4[:, 3]

    in_maps = []
    for cidx in range(NCORES):
        sl = slice(cidx * BC, (cidx + 1) * BC)
        hsp = np.zeros((BC, SB), np.float32)
        hsp[:, 0:R] = h0_full[sl]
        hsp[:, ONES_COL] = 1.0
        hsp[:, X_COL:X_COL + IN] = x[sl]
        hsp[:, H1_OFF:H1_OFF + R] = h1_full[sl]
        hspb = hsp.astype(BF16)
        in_maps.append({
            "w": w8,
            "wb": wb,
            "htci": np.ascontiguousarray(
                hspb.reshape(BC, 8, 128).transpose(2, 1, 0)),
            "hsbi": np.ascontiguousarray(
                hspb.reshape(NB, 128, SB).transpose(1, 0, 2)),
            "c0i": np.ascontiguousarray(
                c0_full[sl].astype(FP16).reshape(NB, 128, R)
                .transpose(1, 0, 2)),
            "c1i": np.ascontiguousarray(
                c1_full[sl].astype(FP16).reshape(NB, 128, R)
                .transpose(1, 0, 2)),
        })
    return in_maps, h1_full


def kernel(x, init_states_input, W_i2h0, b_i2h0, W_h2h0, b_h2h0,
           W_i2h1, b_i2h1, W_h2h1, b_h2h1):
    global LAST_RESULT
    from concourse.bass_utils import run_bass_kernel_spmd

    in_maps, h1_full = prep_inputs(
        x, init_states_input, W_i2h0, b_i2h0, W_h2h0, b_h2h0,
        W_i2h1, b_i2h1, W_h2h1, b_h2h1)

    nc = build_bass(NSTEPS)
    res = run_bass_kernel_spmd(nc, in_maps, list(range(NCORES)), trace=TRACE)
    LAST_RESULT = res

    out = np.empty((B, (NSTEPS + 1) * R), np.float32)
    out[:, 0:R] = h1_full
    for c in range(NCORES):
        out[c * BC:(c + 1) * BC, R:] = \
            np.asarray(res.results[c]["out"]).astype(np.float32)
    return out


# revision 19
# speedup vs baseline: 1.4468x; 1.3982x over previous
"""Bass/Trainium2 kernel for nn_BuildLstmUnrollNet (bf16+fp8 hybrid).

Problem: 2-layer LSTM, unrolled T=11 steps with per-step (non-shared)
weights, B=8192, R=425, IN=20.  Output block t is the last-layer h
*before* step t, so only steps 0..9 need computing.

Strategy (data-parallel over batch, 8 cores x 1024 rows):
  - Step 0 runs its matmuls in bf16 (the initial h/c are raw unbounded
    randn; fp8-quantizing them costs ~6x the tolerable error).  Steps
    1..9 run all matmuls in fp8e4 with perf_mode=DoubleRow (2 K-rows
    per PE cell, 2x throughput): weights are the moving operand
    (pre-scaled x256, g-gate columns additionally x2), transposed
    activations are stationary, 256 K-features per pass.  Post-step-0
    h's are tanh-bounded, so fp8 keeps absmax rel err ~4x under the
    2e-2 gate.
  - Packed batch-major state (bf16): [h0(425) | 1 | x(20) | pad ->512 |
    h1(425) | pad ->1024].  Bias rides the ones column; layer 0
    contracts features 0..511, layer 1 contracts 0..1023 (x rows
    zero-weighted).  The recurrent transpose bounces through DRAM in
    bf16 (2-byte x-bar DMA transpose) into a [128, 8 chunks, 1024]
    K-major buffer; the otherwise-idle GPSIMD/Pool engine then copies
    it to fp8.  DoubleRow pairs adjacent 128-row chunks (pair stride
    1024, 16-aligned, per the s3_lw dual-fp8 ISA restrictions).
  - One single Sigmoid ACT op per (m-tile, layer) covers ALL 1700 gate
    columns: tanh(g) = 2*sigmoid(2g)-1 with the g columns' weights
    doubled on the host; the affine fix-up is a cheap 4x-mode DVE
    tensor_scalar.  PSUM descale (1/256) rides the ACT scale input.
  - Cell math in fp16 on DVE (2x mode), batched over 4-m-tile groups;
    tanh(c) on ACT batched per group.
  - h1 output is stored bf16 (straight from the packed state) and
    upcast on the host.

kernel(**inputs) takes full-size numpy inputs, does the host-side
packing/sharding, runs the same program SPMD on cores 0..7, and
reassembles the full [8192, 4675] fp32 output (block 0 comes straight
from the initial state on the host).
"""

import numpy as np
import ml_dtypes

FP8 = ml_dtypes.float8_e4m3     # TRN float8e4: max normal 240, inf above
BF16 = ml_dtypes.bfloat16
FP16 = np.float16

B = 8192
NCORES = 8
BC = B // NCORES          # batch rows per core (1024)
NB = BC // 128            # m-tiles per core (8)
R = 425
IN = 20
GN = 4 * R                # 1700 gate columns
GNP = 1712                # padded gate cols in the fp8 weight tile (16|GNP)
SW = 256.0                # fp8 weight scale, descaled via ACT scale=1/SW
ONES_COL = R              # 425: ones feature (bias row rides here)
X_COL = R + 1             # 426..445: x features
H1_OFF = 512              # h1 features at 512..936
SB = 1024                 # packed state width = 8 chunks of 128
NSLOT = 6                 # fp8 weight pair-slots: L0 p0,p1 + L1 p0..p3
NCHUNKS = [(0, 512), (512, 512), (1024, 512), (1536, 164)]
GRPS = [(0, 4), (4, 4)]   # m-tile groups for the cell-math pipeline
NSTEPS = 10

# set by test.py to profile; results stashed in LAST_RESULT
TRACE = False
LAST_RESULT = None


def build_bass(n_steps=NSTEPS, finalize=True):
    import concourse.bacc as bacc
    import concourse.mybir as mybir
    import concourse.tile as tile

    f32 = mybir.dt.float32
    bf16 = mybir.dt.bfloat16
    fp16 = mybir.dt.float16
    f8 = mybir.dt.float8e4
    Sig = mybir.ActivationFunctionType.Sigmoid
    Tanh = mybir.ActivationFunctionType.Tanh
    DR = mybir.MatmulPerfMode.DoubleRow
    mult = mybir.AluOpType.mult
    add = mybir.AluOpType.add

    nc = bacc.Bacc()

    w_d = nc.declare_dram_parameter("w", [n_steps, 128, NSLOT, 2, GNP], f8,
                                    False)
    wb_d = nc.declare_dram_parameter("wb", [128, 12, GN], bf16, False)
    htci_d = nc.declare_dram_parameter("htci", [128, 8, BC], bf16, False)
    hsbi_d = nc.declare_dram_parameter("hsbi", [128, NB, SB], bf16, False)
    c0i_d = nc.declare_dram_parameter("c0i", [128, NB, R], fp16, False)
    c1i_d = nc.declare_dram_parameter("c1i", [128, NB, R], fp16, False)
    out_d = nc.declare_dram_parameter("out", [BC, n_steps * R], bf16, True)
    # DRAM bounce buffer for the recurrent transposes
    hd = nc.dram_tensor("hd", [BC, SB], bf16)

    # fp8 pair-slot (within the per-layer weight tile) -> first state chunk
    L0_CH = (0, 2)            # layer-0 pairs: chunks (0,1), (2,3)
    L1_CH = (0, 2, 4, 6)      # layer-1 pairs: chunks (0,1)..(6,7)

    with tile.TileContext(nc) as tc:
        with (
            tc.tile_pool(name="consts", bufs=1) as consts,
            tc.tile_pool(name="wl0p", bufs=2) as wl0p,
            tc.tile_pool(name="wl1p", bufs=2) as wl1p,
            tc.tile_pool(name="w0pool", bufs=1) as w0pool,
            tc.tile_pool(name="gpsum", bufs=2, space="PSUM") as gpsum,
        ):
            # persistent state tiles
            hsb = consts.tile([128, NB, SB], bf16)      # packed batch-major
            htcb = consts.tile([128, 8, BC], bf16)      # transposed bf16
            # fp8 transposed state, version-rotated so every reader is a
            # full wave behind the writer (h0 side needs 3 live versions,
            # h1 side 2)
            ht03a = consts.tile([128, 4, BC], f8)
            ht03b = consts.tile([128, 4, BC], f8)
            ht03c = consts.tile([128, 4, BC], f8)
            ht47a = consts.tile([128, 4, BC], f8)
            ht47b = consts.tile([128, 4, BC], f8)
            ht03 = [ht03a, ht03b, ht03c]
            ht47 = [ht47a, ht47b]
            c0 = consts.tile([128, NB, R], fp16)
            c1 = consts.tile([128, NB, R], fp16)
            ts0 = consts.tile([128, NB, GN], fp16)      # L0 sigmoid outputs
            ts1 = consts.tile([128, NB, GN], fp16)      # L1 sigmoid outputs
            tg16 = consts.tile([128, NB, R], fp16)
            tc16 = consts.tile([128, NB, R], fp16)

            # step-0 bf16 weights: one tile, L0 chunks then L1 chunks
            wb = w0pool.tile([128, 8, GN], bf16)

            # init DMAs on the SP (HWDGE) queue, most-urgent first; step-0
            # bf16 weights + step-1 fp8 weights on the Pool (SWDGE) queue
            for c in range(4):
                nc.sync.dma_start(htcb[:, c], htci_d[:, c])
            nc.gpsimd.dma_start(wb[:, 0:4], wb_d[:, 0:4])
            nc.sync.dma_start(c0[:], c0i_d[:])
            nc.sync.dma_start(hsb[:], hsbi_d[:])
            for c in range(4, 8):
                nc.sync.dma_start(htcb[:, c], htci_d[:, c])
            nc.sync.dma_start(c1[:], c1i_d[:])
            wl0t = {}   # step -> fp8 L0 weight tile [128, 2, 2, GNP]
            wl1t = {}   # step -> fp8 L1 weight tile [128, 4, 2, GNP]
            if n_steps > 1:
                wl0t[1] = wl0p.tile([128, 2, 2, GNP], f8, tag="wl0", name="wl0_1")
                nc.gpsimd.dma_start(wl0t[1][:], w_d[1][:, 0:2])
            if n_steps > 2:
                wl0t[2] = wl0p.tile([128, 2, 2, GNP], f8, tag="wl0", name="wl0_2")
                nc.gpsimd.dma_start(wl0t[2][:], w_d[2][:, 0:2])
            # L1(0)'s h1-side bf16 weights have no WAR on the L0 half:
            # stream them during the prologue
            for q in range(2):
                nc.gpsimd.dma_start(wb[:, 4 + 2 * q:6 + 2 * q],
                                    wb_d[:, 8 + 2 * q:10 + 2 * q])

            # PE warm-up: the HAM clock gate needs ~3.4us of sustained
            # activity before the PE runs at full rate.
            warm = consts.tile([128, 128], bf16)
            nc.vector.memset(warm[:], 0.0)
            wps = gpsum.tile([128, 512], f32, tag="g")
            for i in range(20):
                nc.tensor.matmul(wps[:, 0:128], warm[:], warm[:],
                                 start=True, stop=True)

            def mm_sigma(t, layer, m):
                """Gate matmuls + the single whole-width sigmoid for m.
                fp8 stationary versions: layer-0 of step t reads h0(t-1) =
                ht03[(t-1)%3]; layer-1 of step t reads h0(t) = ht03[t%3]
                and h1(t-1) = ht47[t%2]."""
                g = gpsum.tile([128, GN], f32, tag="g")
                if t == 0:
                    chunks = range(0, 4) if layer == 0 else range(0, 8)
                    nch = len(chunks)
                    for ki, ch in enumerate(chunks):
                        lhsT = htcb[:, ch, m * 128:(m + 1) * 128]
                        for (no, nw) in NCHUNKS:
                            nc.tensor.matmul(
                                g[:, no:no + nw], lhsT,
                                wb[:, ch, no:no + nw],
                                start=(ki == 0), stop=(ki == nch - 1))
                else:
                    if layer == 0:
                        srcs = [(ht03[(t - 1) % 3], 0), (ht03[(t - 1) % 3], 2)]
                        wt = wl0t[t]
                    else:
                        h0v, h1v = ht03[t % 3], ht47[t % 2]
                        srcs = [(h0v, 0), (h0v, 2), (h1v, 0), (h1v, 2)]
                        wt = wl1t[t]
                    for si, (ht, ch) in enumerate(srcs):
                        lhsT = ht[:, ch:ch + 2, m * 128:(m + 1) * 128]
                        for (no, nw) in NCHUNKS:
                            nc.tensor.matmul(
                                g[:, no:no + nw], lhsT,
                                wt[:, si, :, no:no + nw],
                                start=(si == 0), stop=(si == len(srcs) - 1),
                                perf_mode=DR)
                # tanh(g) = 2*sigmoid(2g)-1, g-col weights x2 on the host
                tsl = ts0 if layer == 0 else ts1
                nc.scalar.activation(tsl[:, m], g[:], Sig,
                                     scale=(1.0 if t == 0 else 1.0 / SW))

            def cells_sub(t, layer, s0, sl):
                """fp16 cell math for m-tiles [s0, s0+sl)."""
                ms = slice(s0, s0 + sl)
                tsl = ts0 if layer == 0 else ts1
                cst = c0 if layer == 0 else c1
                hoff = 0 if layer == 0 else H1_OFF
                s_i = tsl[:, ms, 0:R]
                s_f = tsl[:, ms, R:2 * R]
                s_o = tsl[:, ms, 2 * R:3 * R]
                s_g = tsl[:, ms, 3 * R:4 * R]
                nc.vector.tensor_scalar(tg16[:, ms], s_g, 2.0, -1.0,
                                        mult, add)
                nc.vector.tensor_mul(s_g, s_i, tg16[:, ms])       # i*tanh(g)
                nc.vector.tensor_mul(s_i, s_f, cst[:, ms])        # f*c
                nc.vector.tensor_add(cst[:, ms], s_i, s_g)        # c'
                nc.scalar.activation(tc16[:, ms], cst[:, ms], Tanh)
                nc.vector.tensor_mul(hsb[:, ms, hoff:hoff + R], s_o,
                                     tc16[:, ms])

            def cells_dma(t, layer, g0, gl):
                """Group-wide output store, bounce, x-bar transpose and
                Pool fp8ify for the updated packed-state half."""
                msg = slice(g0, g0 + gl)
                rows = slice(g0 * 128, (g0 + gl) * 128)
                hoff = 0 if layer == 0 else H1_OFF
                if layer == 1:
                    nc.sync.dma_start(
                        out_d[rows, t * R:(t + 1) * R]
                        .rearrange("(m p) c -> p m c", p=128),
                        hsb[:, msg, H1_OFF:H1_OFF + R])
                if layer == 0 or t < n_steps - 1:
                    nc.sync.dma_start(
                        hd[rows, hoff:hoff + 512]
                        .rearrange("(m p) c -> p m c", p=128),
                        hsb[:, msg, hoff:hoff + 512])
                    ch0 = 0 if layer == 0 else 4
                    for ch in range(ch0, ch0 + 4):
                        nc.sync.dma_start(
                            out=htcb[:, ch, rows],
                            in_=hd[rows, 128 * ch:128 * ch + 128],
                            transpose=True)
                    dst = ht03[t % 3] if layer == 0 else ht47[(t + 1) % 2]
                    nc.gpsimd.tensor_copy(dst[:, 0:4, rows],
                                          htcb[:, ch0:ch0 + 4, rows])

            def cells(t, layer, g0, gl):
                for s0 in range(g0, g0 + gl, 2):
                    cells_sub(t, layer, s0, min(2, g0 + gl - s0))
                cells_dma(t, layer, g0, gl)

            def block(t, layer, g0, gl):
                """mm+sigma interleaved with the 2-m cell subgroups: the
                first pair's tanh(c) lands at ACT position 4, so the
                bounce->transpose->fp8ify chain starts ~5us earlier."""
                mlist = list(range(g0, g0 + gl))
                for m in mlist[0:3]:
                    mm_sigma(t, layer, m)
                if gl >= 2:
                    cells_sub(t, layer, g0, 2)
                for m in mlist[3:]:
                    mm_sigma(t, layer, m)
                if gl > 2:
                    cells_sub(t, layer, g0 + 2, gl - 2)
                else:
                    pass
                cells_dma(t, layer, g0, gl)

            # prologue: L0(0) and L1(0) matmuls in bf16 (L1(0) must read
            # htcb's h0(0) before L0(1)'s transposes overwrite it), then
            # L0(1) in fp8 (its stationary comes from L0(0)'s cells; this
            # chain stalls once, ~10us)
            for (g0, gl) in GRPS:
                block(0, 0, g0, gl)
            # step-0 L1 h0-side weights overwrite the L0 half of wb
            for q in range(2):
                nc.gpsimd.dma_start(wb[:, 2 * q:2 * q + 2],
                                    wb_d[:, 4 + 2 * q:6 + 2 * q])
            for (g0, gl) in GRPS:
                for m in range(g0, g0 + gl):
                    mm_sigma(0, 1, m)
            if n_steps > 1:
                for (g0, gl) in GRPS:
                    block(1, 0, g0, gl)

            # waves: L1(t) runs alongside L0(t+1).  All matmuls+sigmas are
            # hoisted to the wave front (they depend only on the previous
            # wave's state), so the recurrent bounce->transpose->fp8ify
            # chain hides under a full wave of ACT work; the fp8 stationary
            # ping-pongs by wave parity to kill cross-wave WAR hazards.
            # waves: wave k = {L0(k+2), L1(k)} — every recurrent
            # dependency (h0 and h1 transposed+fp8ified state, weights) is
            # produced at least one full wave before its consumer, so the
            # bounce->transpose->fp8ify chains hide completely
            for k in range(n_steps - 1):
                for (g0, gl) in GRPS:
                    # L1 leads: it reads two-wave-old h0 state, and its h1
                    # chain is consumed at the very start of wave k+1
                    if k >= 1:
                        # k == 0: L1(0) sigmas already ran in the prologue
                        block(k, 1, g0, gl)
                    else:
                        cells(k, 1, g0, gl)
                    if k + 2 <= n_steps - 1:
                        block(k + 2, 0, g0, gl)
                # weight prefetches at the wave tail: their WAR waits (on
                # the previous tile buffer) must not head-of-line-block the
                # Pool FIFO in front of the fp8ify converts
                wl1t[k + 1] = wl1p.tile([128, 4, 2, GNP], f8, tag="wl1",
                                        name=f"wl1_{k+1}")
                for q in range(2):
                    nc.gpsimd.dma_start(
                        wl1t[k + 1][:, 2 * q:2 * q + 2],
                        w_d[k + 1][:, 2 + 2 * q:4 + 2 * q])
                if k + 3 <= n_steps - 1:
                    wl0t[k + 3] = wl0p.tile([128, 2, 2, GNP], f8, tag="wl0",
                                            name=f"wl0_{k+3}")
                    nc.gpsimd.dma_start(wl0t[k + 3][:], w_d[k + 3][:, 0:2])

            # epilogue: layer 1 of the last step
            for (g0, gl) in GRPS:
                if n_steps > 1:
                    block(n_steps - 1, 1, g0, gl)
                else:
                    cells(n_steps - 1, 1, g0, gl)
    if finalize:
        nc.finalize()
    return nc


def prep_inputs(x, init_states_input, W_i2h0, b_i2h0, W_h2h0, b_h2h0,
                W_i2h1, b_i2h1, W_h2h1, b_h2h1, n_steps=NSTEPS):
    """Host-side packing.  Returns (in_maps, h1_init_full)."""
    x = np.asarray(x, np.float32)
    init = np.asarray(init_states_input, np.float32)
    W_i2h0 = np.asarray(W_i2h0, np.float32)[:n_steps]
    b_i2h0 = np.asarray(b_i2h0, np.float32)[:n_steps]
    W_h2h0 = np.asarray(W_h2h0, np.float32)[:n_steps]
    b_h2h0 = np.asarray(b_h2h0, np.float32)[:n_steps]
    W_i2h1 = np.asarray(W_i2h1, np.float32)[:n_steps]
    b_i2h1 = np.asarray(b_i2h1, np.float32)[:n_steps]
    W_h2h1 = np.asarray(W_h2h1, np.float32)[:n_steps]
    b_h2h1 = np.asarray(b_h2h1, np.float32)[:n_steps]

    # per-step K-major weight blocks, rows matching the packed state
    WL0 = np.zeros((n_steps, 512, GN), np.float32)
    WL0[:, 0:R] = W_h2h0.transpose(0, 2, 1)
    WL0[:, ONES_COL] = b_i2h0 + b_h2h0
    WL0[:, X_COL:X_COL + IN] = W_i2h0.transpose(0, 2, 1)
    WL1 = np.zeros((n_steps, SB, GN), np.float32)
    WL1[:, 0:R] = W_i2h1.transpose(0, 2, 1)
    WL1[:, ONES_COL] = b_i2h1 + b_h2h1
    WL1[:, H1_OFF:H1_OFF + R] = W_h2h1.transpose(0, 2, 1)
    for Wx in (WL0, WL1):
        Wx[:, :, 3 * R:] *= 2.0     # g-cols doubled: tanh via sigmoid

    # step-0 bf16 weights: 12 K-chunks of 128 (L0 c0..3, L1 c0..7)
    wb = np.concatenate([WL0[0].reshape(4, 128, GN),
                         WL1[0].reshape(8, 128, GN)], axis=0) \
        .transpose(1, 0, 2)                       # [128, 12, GN]
    wb = np.ascontiguousarray(wb.astype(BF16))

    # fp8 step weights: pair-slot j covers chunks (2j, 2j+1);
    # k = 128*(2j+i) + p  ->  [T, p, slot, i, n], n padded to GNP
    w8f = np.concatenate([
        WL0.reshape(n_steps, 2, 2, 128, GN).transpose(0, 3, 1, 2, 4),
        WL1.reshape(n_steps, 4, 2, 128, GN).transpose(0, 3, 1, 2, 4),
    ], axis=2) * SW                               # [T, 128, 6, 2, GN]
    w8 = np.zeros((n_steps, 128, NSLOT, 2, GNP), FP8)
    w8[..., :GN] = FP8(np.clip(w8f, -240.0, 240.0))

    init4 = init.reshape(B, 4, R)
    h0_full, c0_full = init4[:, 0], init4[:, 1]
    h1_full, c1_full = init4[:, 2], init4[:, 3]

    in_maps = []
    for cidx in range(NCORES):
        sl = slice(cidx * BC, (cidx + 1) * BC)
        hsp = np.zeros((BC, SB), np.float32)
        hsp[:, 0:R] = h0_full[sl]
        hsp[:, ONES_COL] = 1.0
        hsp[:, X_COL:X_COL + IN] = x[sl]
        hsp[:, H1_OFF:H1_OFF + R] = h1_full[sl]
        hspb = hsp.astype(BF16)
        in_maps.append({
            "w": w8,
            "wb": wb,
            "htci": np.ascontiguousarray(
                hspb.reshape(BC, 8, 128).transpose(2, 1, 0)),
            "hsbi": np.ascontiguousarray(
                hspb.reshape(NB, 128, SB).transpose(1, 0, 2)),
            "c0i": np.ascontiguousarray(
                c0_full[sl].astype(FP16).reshape(NB, 128, R)
                .transpose(1, 0, 2)),
            "c1i": np.ascontiguousarray(
                c1_full[sl].astype(FP16).reshape(NB, 128, R)
                .transpose(1, 0, 2)),
        })
    return in_maps, h1_full


def kernel(x, init_states_input, W_i2h0, b_i2h0, W_h2h0, b_h2h0,
           W_i2h1, b_i2h1, W_h2h1, b_h2h1):
    global LAST_RESULT
    from concourse.bass_utils import run_bass_kernel_spmd

    in_maps, h1_full = prep_inputs(
        x, init_states_input, W_i2h0, b_i2h0, W_h2h0, b_h2h0,
        W_i2h1, b_i2h1, W_h2h1, b_h2h1)

    nc = build_bass(NSTEPS)
    res = run_bass_kernel_spmd(nc, in_maps, list(range(NCORES)), trace=TRACE)
    LAST_RESULT = res

    out = np.empty((B, (NSTEPS + 1) * R), np.float32)
    out[:, 0:R] = h1_full
    for c in range(NCORES):
        out[c * BC:(c + 1) * BC, R:] = \
            np.asarray(res.results[c]["out"]).astype(np.float32)
    return out


# revision 22
# speedup vs baseline: 1.4941x; 1.0327x over previous
"""Bass/Trainium2 kernel for nn_BuildLstmUnrollNet (bf16+fp8 hybrid).

Problem: 2-layer LSTM, unrolled T=11 steps with per-step (non-shared)
weights, B=8192, R=425, IN=20.  Output block t is the last-layer h
*before* step t, so only steps 0..9 need computing.

Strategy (data-parallel over batch, 8 cores x 1024 rows):
  - Step 0 runs its matmuls in bf16 (the initial h/c are raw unbounded
    randn; fp8-quantizing them costs ~6x the tolerable error).  Steps
    1..9 run all matmuls in fp8e4 with perf_mode=DoubleRow (2 K-rows
    per PE cell, 2x throughput): weights are the moving operand
    (pre-scaled x256, g-gate columns additionally x2), transposed
    activations are stationary, 256 K-features per pass.  Post-step-0
    h's are tanh-bounded, so fp8 keeps absmax rel err ~4x under the
    2e-2 gate.
  - Packed batch-major state (bf16): [h0(425) | 1 | x(20) | pad ->512 |
    h1(425) | pad ->1024].  Bias rides the ones column; layer 0
    contracts features 0..511, layer 1 contracts 0..1023 (x rows
    zero-weighted).  The recurrent transpose bounces through DRAM in
    bf16 (2-byte x-bar DMA transpose) into a [128, 8 chunks, 1024]
    K-major buffer; the otherwise-idle GPSIMD/Pool engine then copies
    it to fp8.  DoubleRow pairs adjacent 128-row chunks (pair stride
    1024, 16-aligned, per the s3_lw dual-fp8 ISA restrictions).
  - One single Sigmoid ACT op per (m-tile, layer) covers ALL 1700 gate
    columns: tanh(g) = 2*sigmoid(2g)-1 with the g columns' weights
    doubled on the host; the affine fix-up is a cheap 4x-mode DVE
    tensor_scalar.  PSUM descale (1/256) rides the ACT scale input.
  - Cell math in fp16 on DVE (2x mode), batched over 4-m-tile groups;
    tanh(c) on ACT batched per group.
  - h1 output is stored bf16 (straight from the packed state) and
    upcast on the host.

kernel(**inputs) takes full-size numpy inputs, does the host-side
packing/sharding, runs the same program SPMD on cores 0..7, and
reassembles the full [8192, 4675] fp32 output (block 0 comes straight
from the initial state on the host).
"""

import numpy as np
import ml_dtypes

FP8 = ml_dtypes.float8_e4m3     # TRN float8e4: max normal 240, inf above
BF16 = ml_dtypes.bfloat16
FP16 = np.float16

B = 8192
NCORES = 8
BC = B // NCORES          # batch rows per core (1024)
NB = BC // 128            # m-tiles per core (8)
R = 425
IN = 20
GN = 4 * R                # 1700 gate columns
GNP = 1712                # padded gate cols in the fp8 weight tile (16|GNP)
SW = 256.0                # fp8 weight scale, descaled via ACT scale=1/SW
ONES_COL = R              # 425: ones feature (bias row rides here)
X_COL = R + 1             # 426..445: x features
H1_OFF = 512              # h1 features at 512..936
SB = 1024                 # packed state width = 8 chunks of 128
NSLOT = 6                 # fp8 weight pair-slots: L0 p0,p1 + L1 p0..p3
NCHUNKS = [(0, 512), (512, 512), (1024, 512), (1536, 164)]
GRPS = [(0, 4), (4, 4)]   # m-tile groups for the cell-math pipeline
NSTEPS = 10

# set by test.py to profile; results stashed in LAST_RESULT
TRACE = False
LAST_RESULT = None


def build_bass(n_steps=NSTEPS, finalize=True):
    import concourse.bacc as bacc
    import concourse.mybir as mybir
    import concourse.tile as tile

    f32 = mybir.dt.float32
    bf16 = mybir.dt.bfloat16
    fp16 = mybir.dt.float16
    f8 = mybir.dt.float8e4
    Sig = mybir.ActivationFunctionType.Sigmoid
    Tanh = mybir.ActivationFunctionType.Tanh
    DR = mybir.MatmulPerfMode.DoubleRow
    mult = mybir.AluOpType.mult
    add = mybir.AluOpType.add

    nc = bacc.Bacc()

    w_d = nc.declare_dram_parameter("w", [n_steps, 128, NSLOT, 2, GNP], f8,
                                    False)
    wb_d = nc.declare_dram_parameter("wb", [128, 12, GN], bf16, False)
    htci_d = nc.declare_dram_parameter("htci", [128, 8, BC], bf16, False)
    hsbi_d = nc.declare_dram_parameter("hsbi", [128, NB, SB], bf16, False)
    c0i_d = nc.declare_dram_parameter("c0i", [128, NB, R], fp16, False)
    c1i_d = nc.declare_dram_parameter("c1i", [128, NB, R], fp16, False)
    out_d = nc.declare_dram_parameter("out", [BC, n_steps * R], bf16, True)
    # DRAM bounce buffer for the recurrent transposes
    hd = nc.dram_tensor("hd", [BC, SB], bf16)

    # fp8 pair-slot (within the per-layer weight tile) -> first state chunk
    L0_CH = (0, 2)            # layer-0 pairs: chunks (0,1), (2,3)
    L1_CH = (0, 2, 4, 6)      # layer-1 pairs: chunks (0,1)..(6,7)

    with tile.TileContext(nc) as tc:
        with (
            tc.tile_pool(name="consts", bufs=1) as consts,
            tc.tile_pool(name="wl0p", bufs=2) as wl0p,
            tc.tile_pool(name="wl1p", bufs=2) as wl1p,
            tc.tile_pool(name="w0pool", bufs=1) as w0pool,
            tc.tile_pool(name="gpsum", bufs=2, space="PSUM") as gpsum,
        ):
            # persistent state tiles
            hsb = consts.tile([128, NB, SB], bf16)      # packed batch-major
            htcb = consts.tile([128, 8, BC], bf16)      # transposed bf16
            # fp8 transposed state, version-rotated so every reader is a
            # full wave behind the writer (h0 side needs 3 live versions,
            # h1 side 2)
            ht03a = consts.tile([128, 4, BC], f8)
            ht03b = consts.tile([128, 4, BC], f8)
            ht03c = consts.tile([128, 4, BC], f8)
            ht47a = consts.tile([128, 4, BC], f8)
            ht47b = consts.tile([128, 4, BC], f8)
            ht03 = [ht03a, ht03b, ht03c]
            ht47 = [ht47a, ht47b]
            c0 = consts.tile([128, NB, R], fp16)
            c1 = consts.tile([128, NB, R], fp16)
            ts0 = consts.tile([128, NB, GN], fp16)      # L0 sigmoid outputs
            ts1 = consts.tile([128, NB, GN], fp16)      # L1 sigmoid outputs
            tg16 = consts.tile([128, NB, R], fp16)
            tc16 = consts.tile([128, NB, R], fp16)

            # step-0 bf16 weights: one tile, L0 chunks then L1 chunks
            wb = w0pool.tile([128, 8, GN], bf16)

            # init DMAs on the SP (HWDGE) queue, most-urgent first; step-0
            # bf16 weights + step-1 fp8 weights on the Pool (SWDGE) queue
            for c in range(4):
                nc.sync.dma_start(htcb[:, c], htci_d[:, c])
            nc.gpsimd.dma_start(wb[:, 0:4], wb_d[:, 0:4])
            nc.sync.dma_start(c0[:], c0i_d[:])
            nc.sync.dma_start(hsb[:], hsbi_d[:])
            for c in range(4, 8):
                nc.sync.dma_start(htcb[:, c], htci_d[:, c])
            nc.sync.dma_start(c1[:], c1i_d[:])
            # L1(0)'s h1-side bf16 weights have no WAR on the L0 half:
            # stream them during the prologue
            for q in range(2):
                nc.gpsimd.dma_start(wb[:, 4 + 2 * q:6 + 2 * q],
                                    wb_d[:, 8 + 2 * q:10 + 2 * q])

            wl0t = {}   # step -> fp8 L0 weight tile [128, 2, 2, GNP]
            wl1t = {}   # step -> fp8 L1 weight tile [128, 4, 2, GNP]
            if n_steps > 1:
                wl0t[1] = wl0p.tile([128, 2, 2, GNP], f8, tag="wl0", name="wl0_1")
                nc.gpsimd.dma_start(wl0t[1][:], w_d[1][:, 0:2])
            if n_steps > 2:
                wl0t[2] = wl0p.tile([128, 2, 2, GNP], f8, tag="wl0", name="wl0_2")
                nc.gpsimd.dma_start(wl0t[2][:], w_d[2][:, 0:2])

            # PE warm-up: the HAM clock gate needs ~3.4us of sustained
            # activity before the PE runs at full rate.
            warm = consts.tile([128, 128], bf16)
            nc.vector.memset(warm[:], 0.0)
            wps = gpsum.tile([128, 512], f32, tag="g")
            for i in range(20):
                nc.tensor.matmul(wps[:, 0:128], warm[:], warm[:],
                                 start=True, stop=True)

            def mm_sigma(t, layer, m):
                """Gate matmuls + the single whole-width sigmoid for m.
                fp8 stationary versions: layer-0 of step t reads h0(t-1) =
                ht03[(t-1)%3]; layer-1 of step t reads h0(t) = ht03[t%3]
                and h1(t-1) = ht47[t%2]."""
                g = gpsum.tile([128, GN], f32, tag="g")
                if t == 0:
                    chunks = range(0, 4) if layer == 0 else range(0, 8)
                    nch = len(chunks)
                    for ki, ch in enumerate(chunks):
                        lhsT = htcb[:, ch, m * 128:(m + 1) * 128]
                        for (no, nw) in NCHUNKS:
                            nc.tensor.matmul(
                                g[:, no:no + nw], lhsT,
                                wb[:, ch, no:no + nw],
                                start=(ki == 0), stop=(ki == nch - 1))
                else:
                    if layer == 0:
                        srcs = [(ht03[(t - 1) % 3], 0), (ht03[(t - 1) % 3], 2)]
                        wt = wl0t[t]
                    else:
                        h0v, h1v = ht03[t % 3], ht47[t % 2]
                        srcs = [(h0v, 0), (h0v, 2), (h1v, 0), (h1v, 2)]
                        wt = wl1t[t]
                    for si, (ht, ch) in enumerate(srcs):
                        lhsT = ht[:, ch:ch + 2, m * 128:(m + 1) * 128]
                        for (no, nw) in NCHUNKS:
                            nc.tensor.matmul(
                                g[:, no:no + nw], lhsT,
                                wt[:, si, :, no:no + nw],
                                start=(si == 0), stop=(si == len(srcs) - 1),
                                perf_mode=DR)
                # tanh(g) = 2*sigmoid(2g)-1, g-col weights x2 on the host
                tsl = ts0 if layer == 0 else ts1
                nc.scalar.activation(tsl[:, m], g[:], Sig,
                                     scale=(1.0 if t == 0 else 1.0 / SW))

            def cells(t, layer, g0, gl):
                """Batched fp16 cell math + state/output writes for a
                group.  Dead ts gate slots are reused as scratch; the fp8
                stationary convert targets the next wave's parity tile."""
                rows = slice(g0 * 128, (g0 + gl) * 128)
                tsl = ts0 if layer == 0 else ts1
                cst = c0 if layer == 0 else c1
                tcl = tc16
                hoff = 0 if layer == 0 else H1_OFF
                # 2-m-tile sub-batches: the DVE chain for the first pair
                # overlaps the later sigmas, so the bounce fires earlier
                for s0 in range(g0, g0 + gl, 2):
                    ms = slice(s0, min(s0 + 2, g0 + gl))
                    s_i = tsl[:, ms, 0:R]
                    s_f = tsl[:, ms, R:2 * R]
                    s_o = tsl[:, ms, 2 * R:3 * R]
                    s_g = tsl[:, ms, 3 * R:4 * R]
                    nc.vector.tensor_scalar(tg16[:, ms], s_g, 2.0, -1.0,
                                            mult, add)
                    nc.vector.tensor_mul(s_g, s_i, tg16[:, ms])   # i*tanh(g)
                    nc.vector.tensor_mul(s_i, s_f, cst[:, ms])    # f*c
                    nc.vector.tensor_add(cst[:, ms], s_i, s_g)    # c'
                    nc.scalar.activation(tcl[:, ms], cst[:, ms], Tanh)
                    nc.vector.tensor_mul(hsb[:, ms, hoff:hoff + R], s_o,
                                         tcl[:, ms])
                msg = slice(g0, g0 + gl)
                if layer == 1:
                    nc.sync.dma_start(
                        out_d[rows, t * R:(t + 1) * R]
                        .rearrange("(m p) c -> p m c", p=128),
                        hsb[:, msg, H1_OFF:H1_OFF + R])
                if layer == 0 or t < n_steps - 1:
                    # bounce the updated packed half to DRAM, x-bar
                    # transpose back K-major (bf16), then fp8-ify on the
                    # Pool engine for the DoubleRow stationary
                    nc.sync.dma_start(
                        hd[rows, hoff:hoff + 512]
                        .rearrange("(m p) c -> p m c", p=128),
                        hsb[:, msg, hoff:hoff + 512])
                    ch0 = 0 if layer == 0 else 4
                    for ch in range(ch0, ch0 + 4):
                        nc.sync.dma_start(
                            out=htcb[:, ch, rows],
                            in_=hd[rows, 128 * ch:128 * ch + 128],
                            transpose=True)
                    dst = ht03[t % 3] if layer == 0 else ht47[(t + 1) % 2]
                    # two 2-chunk fp8ify ops: downstream DoubleRow pairs can
                    # start after the first pair's chunks land
                    nc.gpsimd.tensor_copy(dst[:, 0:2, rows],
                                          htcb[:, ch0:ch0 + 2, rows])
                    nc.gpsimd.tensor_copy(dst[:, 2:4, rows],
                                          htcb[:, ch0 + 2:ch0 + 4, rows])

            # prologue: L0(0) and L1(0) matmuls in bf16 (L1(0) must read
            # htcb's h0(0) before L0(1)'s transposes overwrite it), then
            # L0(1) in fp8 (its stationary comes from L0(0)'s cells; this
            # chain stalls once, ~10us)
            for (g0, gl) in GRPS:
                for m in range(g0, g0 + gl):
                    mm_sigma(0, 0, m)
                cells(0, 0, g0, gl)
            # step-0 L1 h0-side weights overwrite the L0 half of wb
            for q in range(2):
                nc.gpsimd.dma_start(wb[:, 2 * q:2 * q + 2],
                                    wb_d[:, 4 + 2 * q:6 + 2 * q])
            for (g0, gl) in GRPS:
                for m in range(g0, g0 + gl):
                    mm_sigma(0, 1, m)
            if n_steps > 1:
                for (g0, gl) in GRPS:
                    for m in range(g0, g0 + gl):
                        mm_sigma(1, 0, m)
                    cells(1, 0, g0, gl)

            # waves: L1(t) runs alongside L0(t+1).  All matmuls+sigmas are
            # hoisted to the wave front (they depend only on the previous
            # wave's state), so the recurrent bounce->transpose->fp8ify
            # chain hides under a full wave of ACT work; the fp8 stationary
            # ping-pongs by wave parity to kill cross-wave WAR hazards.
            # waves: wave k = {L0(k+2), L1(k)} — every recurrent
            # dependency (h0 and h1 transposed+fp8ified state, weights) is
            # produced at least one full wave before its consumer, so the
            # bounce->transpose->fp8ify chains hide completely
            for k in range(n_steps - 1):
                for (g0, gl) in GRPS:
                    # L1 leads: it reads two-wave-old h0 state, and its h1
                    # chain is consumed at the very start of wave k+1
                    if k >= 1:
                        # k == 0: L1(0) sigmas already ran in the prologue
                        for m in range(g0, g0 + gl):
                            mm_sigma(k, 1, m)
                    if k + 2 <= n_steps - 1:
                        for m in range(g0, g0 + gl):
                            mm_sigma(k + 2, 0, m)
                    cells(k, 1, g0, gl)
                    if k + 2 <= n_steps - 1:
                        cells(k + 2, 0, g0, gl)
                # weight prefetches at the wave tail: their WAR waits (on
                # the previous tile buffer) must not head-of-line-block the
                # Pool FIFO in front of the fp8ify converts
                wl1t[k + 1] = wl1p.tile([128, 4, 2, GNP], f8, tag="wl1",
                                        name=f"wl1_{k+1}")
                for q in range(2):
                    nc.gpsimd.dma_start(
                        wl1t[k + 1][:, 2 * q:2 * q + 2],
                        w_d[k + 1][:, 2 + 2 * q:4 + 2 * q])
                if k + 3 <= n_steps - 1:
                    wl0t[k + 3] = wl0p.tile([128, 2, 2, GNP], f8, tag="wl0",
                                            name=f"wl0_{k+3}")
                    nc.gpsimd.dma_start(wl0t[k + 3][:], w_d[k + 3][:, 0:2])

            # epilogue: layer 1 of the last step
            for (g0, gl) in GRPS:
                if n_steps > 1:
                    for m in range(g0, g0 + gl):
                        mm_sigma(n_steps - 1, 1, m)
                cells(n_steps - 1, 1, g0, gl)
    if finalize:
        nc.finalize()
    return nc


def prep_inputs(x, init_states_input, W_i2h0, b_i2h0, W_h2h0, b_h2h0,
                W_i2h1, b_i2h1, W_h2h1, b_h2h1, n_steps=NSTEPS):
    """Host-side packing.  Returns (in_maps, h1_init_full)."""
    x = np.asarray(x, np.float32)
    init = np.asarray(init_states_input, np.float32)
    W_i2h0 = np.asarray(W_i2h0, np.float32)[:n_steps]
    b_i2h0 = np.asarray(b_i2h0, np.float32)[:n_steps]
    W_h2h0 = np.asarray(W_h2h0, np.float32)[:n_steps]
    b_h2h0 = np.asarray(b_h2h0, np.float32)[:n_steps]
    W_i2h1 = np.asarray(W_i2h1, np.float32)[:n_steps]
    b_i2h1 = np.asarray(b_i2h1, np.float32)[:n_steps]
    W_h2h1 = np.asarray(W_h2h1, np.float32)[:n_steps]
    b_h2h1 = np.asarray(b_h2h1, np.float32)[:n_steps]

    # per-step K-major weight blocks, rows matching the packed state
    WL0 = np.zeros((n_steps, 512, GN), np.float32)
    WL0[:, 0:R] = W_h2h0.transpose(0, 2, 1)
    WL0[:, ONES_COL] = b_i2h0 + b_h2h0
    WL0[:, X_COL:X_COL + IN] = W_i2h0.transpose(0, 2, 1)
    WL1 = np.zeros((n_steps, SB, GN), np.float32)
    WL1[:, 0:R] = W_i2h1.transpose(0, 2, 1)
    WL1[:, ONES_COL] = b_i2h1 + b_h2h1
    WL1[:, H1_OFF:H1_OFF + R] = W_h2h1.transpose(0, 2, 1)
    for Wx in (WL0, WL1):
        Wx[:, :, 3 * R:] *= 2.0     # g-cols doubled: tanh via sigmoid

    # step-0 bf16 weights: 12 K-chunks of 128 (L0 c0..3, L1 c0..7)
    wb = np.concatenate([WL0[0].reshape(4, 128, GN),
                         WL1[0].reshape(8, 128, GN)], axis=0) \
        .transpose(1, 0, 2)                       # [128, 12, GN]
    wb = np.ascontiguousarray(wb.astype(BF16))

    # fp8 step weights: pair-slot j covers chunks (2j, 2j+1);
    # k = 128*(2j+i) + p  ->  [T, p, slot, i, n], n padded to GNP
    w8f = np.concatenate([
        WL0.reshape(n_steps, 2, 2, 128, GN).transpose(0, 3, 1, 2, 4),
        WL1.reshape(n_steps, 4, 2, 128, GN).transpose(0, 3, 1, 2, 4),
    ], axis=2) * SW                               # [T, 128, 6, 2, GN]
    w8 = np.zeros((n_steps, 128, NSLOT, 2, GNP), FP8)
    w8[..., :GN] = FP8(np.clip(w8f, -240.0, 240.0))

    init4 = init.reshape(B, 4, R)
    h0_full, c0_full = init4[:, 0], init4[:, 1]
    h1_full, c1_full = init4[:, 2], init4[:, 3]

    in_maps = []
    for cidx in range(NCORES):
        sl = slice(cidx * BC, (cidx + 1) * BC)
        hsp = np.zeros((BC, SB), np.float32)
        hsp[:, 0:R] = h0_full[sl]
        hsp[:, ONES_COL] = 1.0
        hsp[:, X_COL:X_COL + IN] = x[sl]
        hsp[:, H1_OFF:H1_OFF + R] = h1_full[sl]
        hspb = hsp.astype(BF16)
        in_maps.append({
            "w": w8,
            "wb": wb,
            "htci": np.ascontiguousarray(
                hspb.reshape(BC, 8, 128).transpose(2, 1, 0)),
            "hsbi": np.ascontiguousarray(
                hspb.reshape(NB, 128, SB).transpose(1, 0, 2)),
            "c0i": np.ascontiguousarray(
                c0_full[sl].astype(FP16).reshape(NB, 128, R)
                .transpose(1, 0, 2)),
            "c1i": np.ascontiguousarray(
                c1_full[sl].astype(FP16).reshape(NB, 128, R)
                .transpose(1, 0, 2)),
        })
    return in_maps, h1_full


def kernel(x, init_states_input, W_i2h0, b_i2h0, W_h2h0, b_h2h0,
           W_i2h1, b_i2h1, W_h2h1, b_h2h1):
    global LAST_RESULT
    from concourse.bass_utils import run_bass_kernel_spmd

    in_maps, h1_full = prep_inputs(
        x, init_states_input, W_i2h0, b_i2h0, W_h2h0, b_h2h0,
        W_i2h1, b_i2h1, W_h2h1, b_h2h1)

    nc = build_bass(NSTEPS)
    res = run_bass_kernel_spmd(nc, in_maps, list(range(NCORES)), trace=TRACE)
    LAST_RESULT = res

    out = np.empty((B, (NSTEPS + 1) * R), np.float32)
    out[:, 0:R] = h1_full
    for c in range(NCORES):
        out[c * BC:(c + 1) * BC, R:] = \
            np.asarray(res.results[c]["out"]).astype(np.float32)
    return out


# revision 23
# speedup vs baseline: 1.5238x; 1.0199x over previous
"""Bass/Trainium2 kernel for nn_BuildLstmUnrollNet (bf16+fp8 hybrid).

Problem: 2-layer LSTM, unrolled T=11 steps with per-step (non-shared)
weights, B=8192, R=425, IN=20.  Output block t is the last-layer h
*before* step t, so only steps 0..9 need computing.

Strategy (data-parallel over batch, 8 cores x 1024 rows):
  - Step 0 runs its matmuls in bf16 (the initial h/c are raw unbounded
    randn; fp8-quantizing them costs ~6x the tolerable error).  Steps
    1..9 run all matmuls in fp8e4 with perf_mode=DoubleRow (2 K-rows
    per PE cell, 2x throughput): weights are the moving operand
    (pre-scaled x256, g-gate columns additionally x2), transposed
    activations are stationary, 256 K-features per pass.  Post-step-0
    h's are tanh-bounded, so fp8 keeps absmax rel err ~4x under the
    2e-2 gate.
  - Packed batch-major state (bf16): [h0(425) | 1 | x(20) | pad ->512 |
    h1(425) | pad ->1024].  Bias rides the ones column; layer 0
    contracts features 0..511, layer 1 contracts 0..1023 (x rows
    zero-weighted).  The recurrent transpose bounces through DRAM in
    bf16 (2-byte x-bar DMA transpose) into a [128, 8 chunks, 1024]
    K-major buffer; the otherwise-idle GPSIMD/Pool engine then copies
    it to fp8.  DoubleRow pairs adjacent 128-row chunks (pair stride
    1024, 16-aligned, per the s3_lw dual-fp8 ISA restrictions).
  - One single Sigmoid ACT op per (m-tile, layer) covers ALL 1700 gate
    columns: tanh(g) = 2*sigmoid(2g)-1 with the g columns' weights
    doubled on the host; the affine fix-up is a cheap 4x-mode DVE
    tensor_scalar.  PSUM descale (1/256) rides the ACT scale input.
  - Cell math in fp16 on DVE (2x mode), batched over 4-m-tile groups;
    tanh(c) on ACT batched per group.
  - h1 output is stored bf16 (straight from the packed state) and
    upcast on the host.

kernel(**inputs) takes full-size numpy inputs, does the host-side
packing/sharding, runs the same program SPMD on cores 0..7, and
reassembles the full [8192, 4675] fp32 output (block 0 comes straight
from the initial state on the host).
"""

import numpy as np
import ml_dtypes

FP8 = ml_dtypes.float8_e4m3     # TRN float8e4: max normal 240, inf above
BF16 = ml_dtypes.bfloat16
FP16 = np.float16

B = 8192
NCORES = 8
BC = B // NCORES          # batch rows per core (1024)
NB = BC // 128            # m-tiles per core (8)
R = 425
IN = 20
GN = 4 * R                # 1700 gate columns
GNP = 1712                # padded gate cols in the fp8 weight tile (16|GNP)
SW = 256.0                # fp8 weight scale, descaled via ACT scale=1/SW
ONES_COL = R              # 425: ones feature (bias row rides here)
X_COL = R + 1             # 426..445: x features
H1_OFF = 512              # h1 features at 512..936
SB = 1024                 # packed state width = 8 chunks of 128
NSLOT = 6                 # fp8 weight pair-slots: L0 p0,p1 + L1 p0..p3
NCHUNKS = [(0, 512), (512, 512), (1024, 512), (1536, 164)]
GRPS = [(0, 4), (4, 4)]   # m-tile groups for the cell-math pipeline
NSTEPS = 10

# set by test.py to profile; results stashed in LAST_RESULT
TRACE = False
LAST_RESULT = None


def build_bass(n_steps=NSTEPS, finalize=True):
    import concourse.bacc as bacc
    import concourse.mybir as mybir
    import concourse.tile as tile

    f32 = mybir.dt.float32
    bf16 = mybir.dt.bfloat16
    fp16 = mybir.dt.float16
    f8 = mybir.dt.float8e4
    Sig = mybir.ActivationFunctionType.Sigmoid
    Tanh = mybir.ActivationFunctionType.Tanh
    DR = mybir.MatmulPerfMode.DoubleRow
    mult = mybir.AluOpType.mult
    add = mybir.AluOpType.add

    nc = bacc.Bacc()

    w_d = nc.declare_dram_parameter("w", [n_steps, 128, NSLOT, 2, GNP], f8,
                                    False)
    wb_d = nc.declare_dram_parameter("wb", [128, 12, GN], bf16, False)
    htci_d = nc.declare_dram_parameter("htci", [128, 8, BC], bf16, False)
    hsbi_d = nc.declare_dram_parameter("hsbi", [128, NB, SB], bf16, False)
    c0i_d = nc.declare_dram_parameter("c0i", [128, NB, R], fp16, False)
    c1i_d = nc.declare_dram_parameter("c1i", [128, NB, R], fp16, False)
    out_d = nc.declare_dram_parameter("out", [BC, n_steps * R], bf16, True)
    # DRAM bounce buffer for the recurrent transposes
    hd = nc.dram_tensor("hd", [BC, SB], bf16)

    # fp8 pair-slot (within the per-layer weight tile) -> first state chunk
    L0_CH = (0, 2)            # layer-0 pairs: chunks (0,1), (2,3)
    L1_CH = (0, 2, 4, 6)      # layer-1 pairs: chunks (0,1)..(6,7)

    with tile.TileContext(nc) as tc:
        with (
            tc.tile_pool(name="consts", bufs=1) as consts,
            tc.tile_pool(name="wl0p", bufs=2) as wl0p,
            tc.tile_pool(name="wl1p", bufs=2) as wl1p,
            tc.tile_pool(name="w0pool", bufs=1) as w0pool,
            tc.tile_pool(name="gpsum", bufs=2, space="PSUM") as gpsum,
        ):
            # persistent state tiles
            hsb = consts.tile([128, NB, SB], bf16)      # packed batch-major
            htcb = consts.tile([128, 8, BC], bf16)      # transposed bf16
            # fp8 transposed state, version-rotated so every reader is a
            # full wave behind the writer (h0 side needs 3 live versions,
            # h1 side 2)
            ht03a = consts.tile([128, 4, BC], f8)
            ht03b = consts.tile([128, 4, BC], f8)
            ht03c = consts.tile([128, 4, BC], f8)
            ht47a = consts.tile([128, 4, BC], f8)
            ht47b = consts.tile([128, 4, BC], f8)
            ht03 = [ht03a, ht03b, ht03c]
            ht47 = [ht47a, ht47b]
            c0 = consts.tile([128, NB, R], fp16)
            c1 = consts.tile([128, NB, R], fp16)
            ts0 = consts.tile([128, NB, GN], fp16)      # L0 sigmoid outputs
            ts1 = consts.tile([128, NB, GN], fp16)      # L1 sigmoid outputs
            tg16 = consts.tile([128, NB, R], fp16)
            tc16 = consts.tile([128, NB, R], fp16)

            # step-0 bf16 weights: one tile, L0 chunks then L1 chunks
            wb = w0pool.tile([128, 8, GN], bf16)

            # init DMAs on the SP (HWDGE) queue, most-urgent first; step-0
            # bf16 weights + step-1 fp8 weights on the Pool (SWDGE) queue
            for c in range(4):
                nc.sync.dma_start(htcb[:, c], htci_d[:, c])
            nc.gpsimd.dma_start(wb[:, 0:4], wb_d[:, 0:4])
            nc.sync.dma_start(c0[:], c0i_d[:])
            nc.sync.dma_start(hsb[:], hsbi_d[:])
            for c in range(4, 8):
                nc.sync.dma_start(htcb[:, c], htci_d[:, c])
            nc.sync.dma_start(c1[:], c1i_d[:])
            # L1(0)'s h1-side bf16 weights have no WAR on the L0 half:
            # stream them during the prologue
            for q in range(2):
                nc.gpsimd.dma_start(wb[:, 4 + 2 * q:6 + 2 * q],
                                    wb_d[:, 8 + 2 * q:10 + 2 * q])

            # L1(0)'s h0-side bf16 weights ride a wl1p buffer (same byte
            # budget as an fp8 wl1 tile) so the load has no WAR on wb
            wbl1h0 = wl1p.tile([128, 4, GN], bf16, tag="wl1")
            for q in range(2):
                nc.gpsimd.dma_start(wbl1h0[:, 2 * q:2 * q + 2],
                                    wb_d[:, 4 + 2 * q:6 + 2 * q])
            wl0t = {}   # step -> fp8 L0 weight tile [128, 2, 2, GNP]
            wl1t = {}   # step -> fp8 L1 weight tile [128, 4, 2, GNP]
            if n_steps > 1:
                wl0t[1] = wl0p.tile([128, 2, 2, GNP], f8, tag="wl0", name="wl0_1")
                nc.gpsimd.dma_start(wl0t[1][:], w_d[1][:, 0:2])
            if n_steps > 2:
                wl0t[2] = wl0p.tile([128, 2, 2, GNP], f8, tag="wl0", name="wl0_2")
                nc.gpsimd.dma_start(wl0t[2][:], w_d[2][:, 0:2])

            # PE warm-up: the HAM clock gate needs ~3.4us of sustained
            # activity before the PE runs at full rate.
            warm = consts.tile([128, 128], bf16)
            nc.vector.memset(warm[:], 0.0)
            wps = gpsum.tile([128, 512], f32, tag="g")
            for i in range(20):
                nc.tensor.matmul(wps[:, 0:128], warm[:], warm[:],
                                 start=True, stop=True)

            def mm_sigma(t, layer, m):
                """Gate matmuls + the single whole-width sigmoid for m.
                fp8 stationary versions: layer-0 of step t reads h0(t-1) =
                ht03[(t-1)%3]; layer-1 of step t reads h0(t) = ht03[t%3]
                and h1(t-1) = ht47[t%2]."""
                g = gpsum.tile([128, GN], f32, tag="g")
                if t == 0:
                    chunks = range(0, 4) if layer == 0 else range(0, 8)
                    nch = len(chunks)
                    for ki, ch in enumerate(chunks):
                        lhsT = htcb[:, ch, m * 128:(m + 1) * 128]
                        if layer == 1 and ch < 4:
                            wsrc = wbl1h0[:, ch]
                        else:
                            wsrc = wb[:, ch]
                        for (no, nw) in NCHUNKS:
                            nc.tensor.matmul(
                                g[:, no:no + nw], lhsT,
                                wsrc[:, no:no + nw],
                                start=(ki == 0), stop=(ki == nch - 1))
                else:
                    if layer == 0:
                        srcs = [(ht03[(t - 1) % 3], 0), (ht03[(t - 1) % 3], 2)]
                        wt = wl0t[t]
                    else:
                        h0v, h1v = ht03[t % 3], ht47[t % 2]
                        srcs = [(h0v, 0), (h0v, 2), (h1v, 0), (h1v, 2)]
                        wt = wl1t[t]
                    for si, (ht, ch) in enumerate(srcs):
                        lhsT = ht[:, ch:ch + 2, m * 128:(m + 1) * 128]
                        for (no, nw) in NCHUNKS:
                            nc.tensor.matmul(
                                g[:, no:no + nw], lhsT,
                                wt[:, si, :, no:no + nw],
                                start=(si == 0), stop=(si == len(srcs) - 1),
                                perf_mode=DR)
                # tanh(g) = 2*sigmoid(2g)-1, g-col weights x2 on the host
                tsl = ts0 if layer == 0 else ts1
                nc.scalar.activation(tsl[:, m], g[:], Sig,
                                     scale=(1.0 if t == 0 else 1.0 / SW))

            def cells(t, layer, g0, gl):
                """Batched fp16 cell math + state/output writes for a
                group.  Dead ts gate slots are reused as scratch; the fp8
                stationary convert targets the next wave's parity tile."""
                rows = slice(g0 * 128, (g0 + gl) * 128)
                tsl = ts0 if layer == 0 else ts1
                cst = c0 if layer == 0 else c1
                tcl = tc16
                hoff = 0 if layer == 0 else H1_OFF
                # 2-m-tile sub-batches: the DVE chain for the first pair
                # overlaps the later sigmas, so the bounce fires earlier
                for s0 in range(g0, g0 + gl, 2):
                    ms = slice(s0, min(s0 + 2, g0 + gl))
                    s_i = tsl[:, ms, 0:R]
                    s_f = tsl[:, ms, R:2 * R]
                    s_o = tsl[:, ms, 2 * R:3 * R]
                    s_g = tsl[:, ms, 3 * R:4 * R]
                    nc.vector.tensor_scalar(tg16[:, ms], s_g, 2.0, -1.0,
                                            mult, add)
                    nc.vector.tensor_mul(s_g, s_i, tg16[:, ms])   # i*tanh(g)
                    nc.vector.tensor_mul(s_i, s_f, cst[:, ms])    # f*c
                    nc.vector.tensor_add(cst[:, ms], s_i, s_g)    # c'
                    nc.scalar.activation(tcl[:, ms], cst[:, ms], Tanh)
                    nc.vector.tensor_mul(hsb[:, ms, hoff:hoff + R], s_o,
                                         tcl[:, ms])
                msg = slice(g0, g0 + gl)
                if layer == 1:
                    nc.sync.dma_start(
                        out_d[rows, t * R:(t + 1) * R]
                        .rearrange("(m p) c -> p m c", p=128),
                        hsb[:, msg, H1_OFF:H1_OFF + R])
                if layer == 0 or t < n_steps - 1:
                    # bounce the updated packed half to DRAM, x-bar
                    # transpose back K-major (bf16), then fp8-ify on the
                    # Pool engine for the DoubleRow stationary
                    nc.sync.dma_start(
                        hd[rows, hoff:hoff + 512]
                        .rearrange("(m p) c -> p m c", p=128),
                        hsb[:, msg, hoff:hoff + 512])
                    ch0 = 0 if layer == 0 else 4
                    for ch in range(ch0, ch0 + 4):
                        nc.sync.dma_start(
                            out=htcb[:, ch, rows],
                            in_=hd[rows, 128 * ch:128 * ch + 128],
                            transpose=True)
                    dst = ht03[t % 3] if layer == 0 else ht47[(t + 1) % 2]
                    # two 2-chunk fp8ify ops: downstream DoubleRow pairs can
                    # start after the first pair's chunks land
                    nc.gpsimd.tensor_copy(dst[:, 0:2, rows],
                                          htcb[:, ch0:ch0 + 2, rows])
                    nc.gpsimd.tensor_copy(dst[:, 2:4, rows],
                                          htcb[:, ch0 + 2:ch0 + 4, rows])

            # prologue: L0(0) and L1(0) matmuls in bf16 (L1(0) must read
            # htcb's h0(0) before L0(1)'s transposes overwrite it), then
            # L0(1) in fp8 (its stationary comes from L0(0)'s cells; this
            # chain stalls once, ~10us)
            for (g0, gl) in GRPS:
                for m in range(g0, g0 + gl):
                    mm_sigma(0, 0, m)
                cells(0, 0, g0, gl)

            for (g0, gl) in GRPS:
                for m in range(g0, g0 + gl):
                    mm_sigma(0, 1, m)
            if n_steps > 1:
                for (g0, gl) in GRPS:
                    for m in range(g0, g0 + gl):
                        mm_sigma(1, 0, m)
                    cells(1, 0, g0, gl)

            # waves: L1(t) runs alongside L0(t+1).  All matmuls+sigmas are
            # hoisted to the wave front (they depend only on the previous
            # wave's state), so the recurrent bounce->transpose->fp8ify
            # chain hides under a full wave of ACT work; the fp8 stationary
            # ping-pongs by wave parity to kill cross-wave WAR hazards.
            # waves: wave k = {L0(k+2), L1(k)} — every recurrent
            # dependency (h0 and h1 transposed+fp8ified state, weights) is
            # produced at least one full wave before its consumer, so the
            # bounce->transpose->fp8ify chains hide completely
            for k in range(n_steps - 1):
                for (g0, gl) in GRPS:
                    # L1 leads: it reads two-wave-old h0 state, and its h1
                    # chain is consumed at the very start of wave k+1
                    if k >= 1:
                        # k == 0: L1(0) sigmas already ran in the prologue
                        for m in range(g0, g0 + gl):
                            mm_sigma(k, 1, m)
                    if k + 2 <= n_steps - 1:
                        for m in range(g0, g0 + gl):
                            mm_sigma(k + 2, 0, m)
                    cells(k, 1, g0, gl)
                    if k + 2 <= n_steps - 1:
                        cells(k + 2, 0, g0, gl)
                # weight prefetches at the wave tail: their WAR waits (on
                # the previous tile buffer) must not head-of-line-block the
                # Pool FIFO in front of the fp8ify converts
                wl1t[k + 1] = wl1p.tile([128, 4, 2, GNP], f8, tag="wl1",
                                        name=f"wl1_{k+1}")
                for q in range(2):
                    nc.gpsimd.dma_start(
                        wl1t[k + 1][:, 2 * q:2 * q + 2],
                        w_d[k + 1][:, 2 + 2 * q:4 + 2 * q])
                if k + 3 <= n_steps - 1:
                    wl0t[k + 3] = wl0p.tile([128, 2, 2, GNP], f8, tag="wl0",
                                            name=f"wl0_{k+3}")
                    nc.gpsimd.dma_start(wl0t[k + 3][:], w_d[k + 3][:, 0:2])

            # epilogue: layer 1 of the last step
            for (g0, gl) in GRPS:
                if n_steps > 1:
                    for m in range(g0, g0 + gl):
                        mm_sigma(n_steps - 1, 1, m)
                cells(n_steps - 1, 1, g0, gl)
    if finalize:
        nc.finalize()
    return nc


def prep_inputs(x, init_states_input, W_i2h0, b_i2h0, W_h2h0, b_h2h0,
                W_i2h1, b_i2h1, W_h2h1, b_h2h1, n_steps=NSTEPS):
    """Host-side packing.  Returns (in_maps, h1_init_full)."""
    x = np.asarray(x, np.float32)
    init = np.asarray(init_states_input, np.float32)
    W_i2h0 = np.asarray(W_i2h0, np.float32)[:n_steps]
    b_i2h0 = np.asarray(b_i2h0, np.float32)[:n_steps]
    W_h2h0 = np.asarray(W_h2h0, np.float32)[:n_steps]
    b_h2h0 = np.asarray(b_h2h0, np.float32)[:n_steps]
    W_i2h1 = np.asarray(W_i2h1, np.float32)[:n_steps]
    b_i2h1 = np.asarray(b_i2h1, np.float32)[:n_steps]
    W_h2h1 = np.asarray(W_h2h1, np.float32)[:n_steps]
    b_h2h1 = np.asarray(b_h2h1, np.float32)[:n_steps]

    # per-step K-major weight blocks, rows matching the packed state
    WL0 = np.zeros((n_steps, 512, GN), np.float32)
    WL0[:, 0:R] = W_h2h0.transpose(0, 2, 1)
    WL0[:, ONES_COL] = b_i2h0 + b_h2h0
    WL0[:, X_COL:X_COL + IN] = W_i2h0.transpose(0, 2, 1)
    WL1 = np.zeros((n_steps, SB, GN), np.float32)
    WL1[:, 0:R] = W_i2h1.transpose(0, 2, 1)
    WL1[:, ONES_COL] = b_i2h1 + b_h2h1
    WL1[:, H1_OFF:H1_OFF + R] = W_h2h1.transpose(0, 2, 1)
    for Wx in (WL0, WL1):
        Wx[:, :, 3 * R:] *= 2.0     # g-cols doubled: tanh via sigmoid

    # step-0 bf16 weights: 12 K-chunks of 128 (L0 c0..3, L1 c0..7)
    wb = np.concatenate([WL0[0].reshape(4, 128, GN),
                         WL1[0].reshape(8, 128, GN)], axis=0) \
        .transpose(1, 0, 2)                       # [128, 12, GN]
    wb = np.ascontiguousarray(wb.astype(BF16))

    # fp8 step weights: pair-slot j covers chunks (2j, 2j+1);
    # k = 128*(2j+i) + p  ->  [T, p, slot, i, n], n padded to GNP
    w8f = np.concatenate([
        WL0.reshape(n_steps, 2, 2, 128, GN).transpose(0, 3, 1, 2, 4),
        WL1.reshape(n_steps, 4, 2, 128, GN).transpose(0, 3, 1, 2, 4),
    ], axis=2) * SW                               # [T, 128, 6, 2, GN]
    w8 = np.zeros((n_steps, 128, NSLOT, 2, GNP), FP8)
    w8[..., :GN] = FP8(np.clip(w8f, -240.0, 240.0))

    init4 = init.reshape(B, 4, R)
    h0_full, c0_full = init4[:, 0], init4[:, 1]
    h1_full, c1_full = init4[:, 2], init4[:, 3]

    in_maps = []
    for cidx in range(NCORES):
        sl = slice(cidx * BC, (cidx + 1) * BC)
        hsp = np.zeros((BC, SB), np.float32)
        hsp[:, 0:R] = h0_full[sl]
        hsp[:, ONES_COL] = 1.0
        hsp[:, X_COL:X_COL + IN] = x[sl]
        hsp[:, H1_OFF:H1_OFF + R] = h1_full[sl]
        hspb = hsp.astype(BF16)
        in_maps.append({
            "w": w8,
            "wb": wb,
            "htci": np.ascontiguousarray(
                hspb.reshape(BC, 8, 128).transpose(2, 1, 0)),
            "hsbi": np.ascontiguousarray(
                hspb.reshape(NB, 128, SB).transpose(1, 0, 2)),
            "c0i": np.ascontiguousarray(
                c0_full[sl].astype(FP16).reshape(NB, 128, R)
                .transpose(1, 0, 2)),
            "c1i": np.ascontiguousarray(
                c1_full[sl].astype(FP16).reshape(NB, 128, R)
                .transpose(1, 0, 2)),
        })
    return in_maps, h1_full


def kernel(x, init_states_input, W_i2h0, b_i2h0, W_h2h0, b_h2h0,
           W_i2h1, b_i2h1, W_h2h1, b_h2h1):
    global LAST_RESULT
    from concourse.bass_utils import run_bass_kernel_spmd

    in_maps, h1_full = prep_inputs(
        x, init_states_input, W_i2h0, b_i2h0, W_h2h0, b_h2h0,
        W_i2h1, b_i2h1, W_h2h1, b_h2h1)

    nc = build_bass(NSTEPS)
    res = run_bass_kernel_spmd(nc, in_maps, list(range(NCORES)), trace=TRACE)
    LAST_RESULT = res

    out = np.empty((B, (NSTEPS + 1) * R), np.float32)
    out[:, 0:R] = h1_full
    for c in range(NCORES):
        out[c * BC:(c + 1) * BC, R:] = \
            np.asarray(res.results[c]["out"]).astype(np.float32)
    return out


# revision 35
# speedup vs baseline: 1.5994x; 1.0497x over previous
"""Bass/Trainium2 kernel for nn_BuildLstmUnrollNet (bf16+fp8 hybrid).

Problem: 2-layer LSTM, unrolled T=11 steps with per-step (non-shared)
weights, B=8192, R=425, IN=20.  Output block t is the last-layer h
*before* step t, so only steps 0..9 need computing.

Strategy (data-parallel over batch, 8 cores x 1024 rows):
  - Step 0 runs its matmuls in bf16 (the initial h/c are raw unbounded
    randn; fp8-quantizing them costs ~6x the tolerable error).  Steps
    1..9 run all matmuls in fp8e4 with perf_mode=DoubleRow (2 K-rows
    per PE cell, 2x throughput): weights are the moving operand
    (pre-scaled x256, g-gate columns additionally x2), transposed
    activations are stationary, 256 K-features per pass.  Post-step-0
    h's are tanh-bounded, so fp8 keeps absmax rel err ~4x under the
    2e-2 gate.
  - Packed batch-major state (bf16): [h0(425) | 1 | x(20) | pad ->512 |
    h1(425) | pad ->1024].  Bias rides the ones column; layer 0
    contracts features 0..511, layer 1 contracts 0..1023 (x rows
    zero-weighted).  The recurrent transpose bounces through DRAM in
    bf16 (2-byte x-bar DMA transpose) into a [128, 8 chunks, 1024]
    K-major buffer; the otherwise-idle GPSIMD/Pool engine then copies
    it to fp8.  DoubleRow pairs adjacent 128-row chunks (pair stride
    1024, 16-aligned, per the s3_lw dual-fp8 ISA restrictions).
  - One single Sigmoid ACT op per (m-tile, layer) covers ALL 1700 gate
    columns: tanh(g) = 2*sigmoid(2g)-1 with the g columns' weights
    doubled on the host; the affine fix-up is a cheap 4x-mode DVE
    tensor_scalar.  PSUM descale (1/256) rides the ACT scale input.
  - Cell math in fp16 on DVE (2x mode), batched over 4-m-tile groups;
    tanh(c) on ACT batched per group.
  - h1 output is stored bf16 (straight from the packed state) and
    upcast on the host.

kernel(**inputs) takes full-size numpy inputs, does the host-side
packing/sharding, runs the same program SPMD on cores 0..7, and
reassembles the full [8192, 4675] fp32 output (block 0 comes straight
from the initial state on the host).
"""

import numpy as np
import ml_dtypes

FP8 = ml_dtypes.float8_e4m3     # TRN float8e4: max normal 240, inf above
BF16 = ml_dtypes.bfloat16
FP16 = np.float16

B = 8192
NCORES = 8
BC = B // NCORES          # batch rows per core (1024)
NB = BC // 128            # m-tiles per core (8)
R = 425
IN = 20
GN = 4 * R                # 1700 gate columns
GNP = 1712                # padded gate cols in the fp8 weight tile (16|GNP)
SW = 256.0                # fp8 weight scale, descaled via ACT scale=1/SW
ONES_COL = R              # 425: ones feature (bias row rides here)
X_COL = R + 1             # 426..445: x features
H1_OFF = 512              # h1 features at 512..936
SB = 1024                 # packed state width = 8 chunks of 128
NSLOT = 6                 # fp8 weight pair-slots: L0 p0,p1 + L1 p0..p3
NCHUNKS = [(0, 512), (512, 512), (1024, 512), (1536, 164)]
GRPS = [(0, 3), (3, 5)]   # m-tile groups for the cell-math pipeline
NSTEPS = 10

# set by test.py to profile; results stashed in LAST_RESULT
TRACE = False
LAST_RESULT = None


def build_bass(n_steps=NSTEPS, finalize=True):
    import concourse.bacc as bacc
    import concourse.mybir as mybir
    import concourse.tile as tile

    f32 = mybir.dt.float32
    bf16 = mybir.dt.bfloat16
    fp16 = mybir.dt.float16
    f8 = mybir.dt.float8e4
    Sig = mybir.ActivationFunctionType.Sigmoid
    Tanh = mybir.ActivationFunctionType.Tanh
    DR = mybir.MatmulPerfMode.DoubleRow
    mult = mybir.AluOpType.mult
    add = mybir.AluOpType.add

    nc = bacc.Bacc()

    w_d = nc.declare_dram_parameter("w", [n_steps, 128, NSLOT, 2, GNP], f8,
                                    False)
    wb_d = nc.declare_dram_parameter("wb", [128, 12, GN], bf16, False)
    htci_d = nc.declare_dram_parameter("htci", [128, 8, BC], bf16, False)
    hsbi_d = nc.declare_dram_parameter("hsbi", [128, NB, SB], bf16, False)
    c0i_d = nc.declare_dram_parameter("c0i", [128, NB, R], fp16, False)
    c1i_d = nc.declare_dram_parameter("c1i", [128, NB, R], fp16, False)
    out_d = nc.declare_dram_parameter("out", [BC, n_steps * R], bf16, True)
    # DRAM bounce buffer for the recurrent transposes
    hd = nc.dram_tensor("hd", [BC, SB], bf16)

    # fp8 pair-slot (within the per-layer weight tile) -> first state chunk
    L0_CH = (0, 2)            # layer-0 pairs: chunks (0,1), (2,3)
    L1_CH = (0, 2, 4, 6)      # layer-1 pairs: chunks (0,1)..(6,7)

    with tile.TileContext(nc) as tc:
        with (
            tc.tile_pool(name="consts", bufs=1) as consts,
            tc.tile_pool(name="wl0p", bufs=2) as wl0p,
            tc.tile_pool(name="wl1p", bufs=2) as wl1p,
            tc.tile_pool(name="w0pool", bufs=1) as w0pool,
            tc.tile_pool(name="gpsum", bufs=2, space="PSUM") as gpsum,
        ):
            # persistent state tiles
            hsb = consts.tile([128, NB, SB], bf16)      # packed batch-major
            htcb = consts.tile([128, 8, BC], bf16)      # transposed bf16
            # fp8 transposed state, version-rotated so every reader is a
            # full wave behind the writer (h0 side needs 3 live versions,
            # h1 side 2)
            ht03a = consts.tile([128, 4, BC], f8)
            ht03b = consts.tile([128, 4, BC], f8)
            ht03c = consts.tile([128, 4, BC], f8)
            ht47a = consts.tile([128, 4, BC], f8)
            ht47b = consts.tile([128, 4, BC], f8)
            ht03 = [ht03a, ht03b, ht03c]
            ht47 = [ht47a, ht47b]
            c0 = consts.tile([128, NB, R], fp16)
            c1 = consts.tile([128, NB, R], fp16)
            ts0 = consts.tile([128, NB, GN], fp16)      # L0 sigmoid outputs
            ts1 = consts.tile([128, NB, GN], fp16)      # L1 sigmoid outputs
            tg16 = consts.tile([128, NB, R], fp16)
            tc16 = consts.tile([128, NB, R], fp16)

            # step-0 bf16 weights: one tile, L0 chunks then L1 chunks
            wb = w0pool.tile([128, 8, GN], bf16)

            # init DMAs on the SP (HWDGE) queue, most-urgent first; step-0
            # bf16 weights + step-1 fp8 weights on the Pool (SWDGE) queue
            for c in range(4):
                nc.sync.dma_start(htcb[:, c], htci_d[:, c])
            nc.gpsimd.dma_start(wb[:, 0:4], wb_d[:, 0:4])
            nc.sync.dma_start(c0[:], c0i_d[:])
            nc.sync.dma_start(hsb[:], hsbi_d[:])
            for c in range(4, 8):
                nc.sync.dma_start(htcb[:, c], htci_d[:, c])
            nc.sync.dma_start(c1[:], c1i_d[:])
            # L1(0)'s h1-side bf16 weights have no WAR on the L0 half:
            # stream them during the prologue
            for q in range(2):
                nc.gpsimd.dma_start(wb[:, 4 + 2 * q:6 + 2 * q],
                                    wb_d[:, 8 + 2 * q:10 + 2 * q])

            # L1(0)'s h0-side bf16 weights ride a wl1p buffer (same byte
            # budget as an fp8 wl1 tile) so the load has no WAR on wb
            wbl1h0 = wl1p.tile([128, 4, GN], bf16, tag="wl1")
            for q in range(2):
                nc.gpsimd.dma_start(wbl1h0[:, 2 * q:2 * q + 2],
                                    wb_d[:, 4 + 2 * q:6 + 2 * q])
            wl0t = {}   # step -> fp8 L0 weight tile [128, 2, 2, GNP]
            wl1t = {}   # step -> fp8 L1 weight tile [128, 4, 2, GNP]
            if n_steps > 1:
                wl0t[1] = wl0p.tile([128, 2, 2, GNP], f8, tag="wl0", name="wl0_1")
                nc.gpsimd.dma_start(wl0t[1][:], w_d[1][:, 0:2])
            if n_steps > 2:
                wl0t[2] = wl0p.tile([128, 2, 2, GNP], f8, tag="wl0", name="wl0_2")
                nc.gpsimd.dma_start(wl0t[2][:], w_d[2][:, 0:2])

            # PE warm-up: the HAM clock gate needs ~3.4us of sustained
            # activity before the PE runs at full rate.
            warm = consts.tile([128, 128], bf16)
            nc.vector.memset(warm[:], 0.0)
            wps = gpsum.tile([128, 512], f32, tag="g")
            for i in range(20):
                nc.tensor.matmul(wps[:, 0:128], warm[:], warm[:],
                                 start=True, stop=True)

            def mm_sigma(t, layer, m):
                """Gate matmuls + the single whole-width sigmoid for m.
                fp8 stationary versions: layer-0 of step t reads h0(t-1) =
                ht03[(t-1)%3]; layer-1 of step t reads h0(t) = ht03[t%3]
                and h1(t-1) = ht47[t%2]."""
                g = gpsum.tile([128, GN], f32, tag="g")
                if t == 0:
                    chunks = range(0, 4) if layer == 0 else range(0, 8)
                    nch = len(chunks)
                    for ki, ch in enumerate(chunks):
                        lhsT = htcb[:, ch, m * 128:(m + 1) * 128]
                        if layer == 1 and ch < 4:
                            wsrc = wbl1h0[:, ch]
                        else:
                            wsrc = wb[:, ch]
                        for (no, nw) in NCHUNKS:
                            nc.tensor.matmul(
                                g[:, no:no + nw], lhsT,
                                wsrc[:, no:no + nw],
                                start=(ki == 0), stop=(ki == nch - 1))
                else:
                    if layer == 0:
                        srcs = [(ht03[(t - 1) % 3], 0), (ht03[(t - 1) % 3], 2)]
                        wt = wl0t[t]
                    else:
                        h0v, h1v = ht03[t % 3], ht47[t % 2]
                        srcs = [(h0v, 0), (h0v, 2), (h1v, 0), (h1v, 2)]
                        wt = wl1t[t]
                    for si, (ht, ch) in enumerate(srcs):
                        lhsT = ht[:, ch:ch + 2, m * 128:(m + 1) * 128]
                        for (no, nw) in NCHUNKS:
                            nc.tensor.matmul(
                                g[:, no:no + nw], lhsT,
                                wt[:, si, :, no:no + nw],
                                start=(si == 0), stop=(si == len(srcs) - 1),
                                perf_mode=DR)
                # tanh(g) = 2*sigmoid(2g)-1, g-col weights x2 on the host
                tsl = ts0 if layer == 0 else ts1
                nc.scalar.activation(tsl[:, m], g[:], Sig,
                                     scale=(1.0 if t == 0 else 1.0 / SW))

            def cells(t, layer, g0, gl):
                """Batched fp16 cell math + state/output writes for a
                group.  Dead ts gate slots are reused as scratch; the fp8
                stationary convert targets the next wave's parity tile."""
                rows = slice(g0 * 128, (g0 + gl) * 128)
                tsl = ts0 if layer == 0 else ts1
                cst = c0 if layer == 0 else c1
                tcl = tc16
                hoff = 0 if layer == 0 else H1_OFF
                # 2-m-tile sub-batches: the DVE chain for the first pair
                # overlaps the later sigmas, so the bounce fires earlier
                for s0 in range(g0, g0 + gl, 2):
                    ms = slice(s0, min(s0 + 2, g0 + gl))
                    s_i = tsl[:, ms, 0:R]
                    s_f = tsl[:, ms, R:2 * R]
                    s_o = tsl[:, ms, 2 * R:3 * R]
                    s_g = tsl[:, ms, 3 * R:4 * R]
                    nc.vector.tensor_scalar(tg16[:, ms], s_g, 2.0, -1.0,
                                            mult, add)
                    nc.vector.tensor_mul(s_g, s_i, tg16[:, ms])   # i*tanh(g)
                    nc.vector.tensor_mul(s_i, s_f, cst[:, ms])    # f*c
                    nc.vector.tensor_add(cst[:, ms], s_i, s_g)    # c'
                    nc.scalar.activation(tcl[:, ms], cst[:, ms], Tanh)
                    nc.vector.tensor_mul(hsb[:, ms, hoff:hoff + R], s_o,
                                         tcl[:, ms])
                msg = slice(g0, g0 + gl)
                if layer == 1:
                    nc.sync.dma_start(
                        out_d[rows, t * R:(t + 1) * R]
                        .rearrange("(m p) c -> p m c", p=128),
                        hsb[:, msg, H1_OFF:H1_OFF + R])
                if layer == 0 or t < n_steps - 1:
                    # bounce the updated packed half to DRAM, x-bar
                    # transpose back K-major (bf16), then fp8-ify on the
                    # Pool engine for the DoubleRow stationary
                    nc.sync.dma_start(
                        hd[rows, hoff:hoff + 512]
                        .rearrange("(m p) c -> p m c", p=128),
                        hsb[:, msg, hoff:hoff + 512])
                    ch0 = 0 if layer == 0 else 4
                    for ch in range(ch0, ch0 + 4):
                        nc.sync.dma_start(
                            out=htcb[:, ch, rows],
                            in_=hd[rows, 128 * ch:128 * ch + 128],
                            transpose=True)
                    dst = ht03[t % 3] if layer == 0 else ht47[(t + 1) % 2]
                    # two 2-chunk fp8ify ops: downstream DoubleRow pairs can
                    # start after the first pair's chunks land
                    nc.gpsimd.tensor_copy(dst[:, 0:2, rows],
                                          htcb[:, ch0:ch0 + 2, rows])
                    nc.gpsimd.tensor_copy(dst[:, 2:4, rows],
                                          htcb[:, ch0 + 2:ch0 + 4, rows])

            # prologue: L0(0) and L1(0) matmuls in bf16 (L1(0) must read
            # htcb's h0(0) before L0(1)'s transposes overwrite it), then
            # L0(1) in fp8 (its stationary comes from L0(0)'s cells; this
            # chain stalls once, ~10us)
            for (g0, gl) in GRPS:
                for m in range(g0, g0 + gl):
                    mm_sigma(0, 0, m)
                cells(0, 0, g0, gl)

            for (g0, gl) in GRPS:
                for m in range(g0, g0 + gl):
                    mm_sigma(0, 1, m)
            if n_steps > 1:
                for (g0, gl) in GRPS:
                    for m in range(g0, g0 + gl):
                        mm_sigma(1, 0, m)
                    cells(1, 0, g0, gl)

            # waves: L1(t) runs alongside L0(t+1).  All matmuls+sigmas are
            # hoisted to the wave front (they depend only on the previous
            # wave's state), so the recurrent bounce->transpose->fp8ify
            # chain hides under a full wave of ACT work; the fp8 stationary
            # ping-pongs by wave parity to kill cross-wave WAR hazards.
            # waves: wave k = {L0(k+2), L1(k)} — every recurrent
            # dependency (h0 and h1 transposed+fp8ified state, weights) is
            # produced at least one full wave before its consumer, so the
            # bounce->transpose->fp8ify chains hide completely
            for k in range(n_steps - 1):
                for (g0, gl) in GRPS:
                    # L1 leads: it reads two-wave-old h0 state, and its h1
                    # chain is consumed at the very start of wave k+1
                    if k >= 1:
                        # k == 0: L1(0) sigmas already ran in the prologue
                        for m in range(g0, g0 + gl):
                            mm_sigma(k, 1, m)
                    if k + 2 <= n_steps - 1:
                        for m in range(g0, g0 + gl):
                            mm_sigma(k + 2, 0, m)
                    cells(k, 1, g0, gl)
                    if k + 2 <= n_steps - 1:
                        cells(k + 2, 0, g0, gl)
                # weight prefetches at the wave tail: their WAR waits (on
                # the previous tile buffer) must not head-of-line-block the
                # Pool FIFO in front of the fp8ify converts
                wl1t[k + 1] = wl1p.tile([128, 4, 2, GNP], f8, tag="wl1",
                                        name=f"wl1_{k+1}")
                for q in range(2):
                    nc.gpsimd.dma_start(
                        wl1t[k + 1][:, 2 * q:2 * q + 2],
                        w_d[k + 1][:, 2 + 2 * q:4 + 2 * q])
                if k + 3 <= n_steps - 1:
                    wl0t[k + 3] = wl0p.tile([128, 2, 2, GNP], f8, tag="wl0",
                                            name=f"wl0_{k+3}")
                    nc.gpsimd.dma_start(wl0t[k + 3][:], w_d[k + 3][:, 0:2])

            # epilogue: layer 1 of the last step
            for (g0, gl) in GRPS:
                if n_steps > 1:
                    for m in range(g0, g0 + gl):
                        mm_sigma(n_steps - 1, 1, m)
                cells(n_steps - 1, 1, g0, gl)
    if finalize:
        nc.finalize()
    return nc


def prep_inputs(x, init_states_input, W_i2h0, b_i2h0, W_h2h0, b_h2h0,
                W_i2h1, b_i2h1, W_h2h1, b_h2h1, n_steps=NSTEPS):
    """Host-side packing.  Returns (in_maps, h1_init_full)."""
    x = np.asarray(x, np.float32)
    init = np.asarray(init_states_input, np.float32)
    W_i2h0 = np.asarray(W_i2h0, np.float32)[:n_steps]
    b_i2h0 = np.asarray(b_i2h0, np.float32)[:n_steps]
    W_h2h0 = np.asarray(W_h2h0, np.float32)[:n_steps]
    b_h2h0 = np.asarray(b_h2h0, np.float32)[:n_steps]
    W_i2h1 = np.asarray(W_i2h1, np.float32)[:n_steps]
    b_i2h1 = np.asarray(b_i2h1, np.float32)[:n_steps]
    W_h2h1 = np.asarray(W_h2h1, np.float32)[:n_steps]
    b_h2h1 = np.asarray(b_h2h1, np.float32)[:n_steps]

    # per-step K-major weight blocks, rows matching the packed state
    WL0 = np.zeros((n_steps, 512, GN), np.float32)
    WL0[:, 0:R] = W_h2h0.transpose(0, 2, 1)
    WL0[:, ONES_COL] = b_i2h0 + b_h2h0
    WL0[:, X_COL:X_COL + IN] = W_i2h0.transpose(0, 2, 1)
    WL1 = np.zeros((n_steps, SB, GN), np.float32)
    WL1[:, 0:R] = W_i2h1.transpose(0, 2, 1)
    WL1[:, ONES_COL] = b_i2h1 + b_h2h1
    WL1[:, H1_OFF:H1_OFF + R] = W_h2h1.transpose(0, 2, 1)
    for Wx in (WL0, WL1):
        Wx[:, :, 3 * R:] *= 2.0     # g-cols doubled: tanh via sigmoid

    # step-0 bf16 weights: 12 K-chunks of 128 (L0 c0..3, L1 c0..7)
    wb = np.concatenate([WL0[0].reshape(4, 128, GN),
                         WL1[0].reshape(8, 128, GN)], axis=0) \
        .transpose(1, 0, 2)                       # [128, 12, GN]
    wb = np.ascontiguousarray(wb.astype(BF16))

    # fp8 step weights: pair-slot j covers chunks (2j, 2j+1);
    # k = 128*(2j+i) + p  ->  [T, p, slot, i, n], n padded to GNP
    w8f = np.concatenate([
        WL0.reshape(n_steps, 2, 2, 128, GN).transpose(0, 3, 1, 2, 4),
        WL1.reshape(n_steps, 4, 2, 128, GN).transpose(0, 3, 1, 2, 4),
    ], axis=2) * SW                               # [T, 128, 6, 2, GN]
    w8 = np.zeros((n_steps, 128, NSLOT, 2, GNP), FP8)
    w8[..., :GN] = FP8(np.clip(w8f, -240.0, 240.0))

    init4 = init.reshape(B, 4, R)
    h0_full, c0_full = init4[:, 0], init4[:, 1]
    h1_full, c1_full = init4[:, 2], init4[:, 3]

    in_maps = []
    for cidx in range(NCORES):
        sl = slice(cidx * BC, (cidx + 1) * BC)
        hsp = np.zeros((BC, SB), np.float32)
        hsp[:, 0:R] = h0_full[sl]
        hsp[:, ONES_COL] = 1.0
        hsp[:, X_COL:X_COL + IN] = x[sl]
        hsp[:, H1_OFF:H1_OFF + R] = h1_full[sl]
        hspb = hsp.astype(BF16)
        in_maps.append({
            "w": w8,
            "wb": wb,
            "htci": np.ascontiguousarray(
                hspb.reshape(BC, 8, 128).transpose(2, 1, 0)),
            "hsbi": np.ascontiguousarray(
                hspb.reshape(NB, 128, SB).transpose(1, 0, 2)),
            "c0i": np.ascontiguousarray(
                c0_full[sl].astype(FP16).reshape(NB, 128, R)
                .transpose(1, 0, 2)),
            "c1i": np.ascontiguousarray(
                c1_full[sl].astype(FP16).reshape(NB, 128, R)
                .transpose(1, 0, 2)),
        })
    return in_maps, h1_full


def kernel(x, init_states_input, W_i2h0, b_i2h0, W_h2h0, b_h2h0,
           W_i2h1, b_i2h1, W_h2h1, b_h2h1):
    global LAST_RESULT
    from concourse.bass_utils import run_bass_kernel_spmd

    in_maps, h1_full = prep_inputs(
        x, init_states_input, W_i2h0, b_i2h0, W_h2h0, b_h2h0,
        W_i2h1, b_i2h1, W_h2h1, b_h2h1)

    nc = build_bass(NSTEPS)
    res = run_bass_kernel_spmd(nc, in_maps, list(range(NCORES)), trace=TRACE)
    LAST_RESULT = res

    out = np.empty((B, (NSTEPS + 1) * R), np.float32)
    out[:, 0:R] = h1_full
    for c in range(NCORES):
        out[c * BC:(c + 1) * BC, R:] = \
            np.asarray(res.results[c]["out"]).astype(np.float32)
    return out


# revision 40
# speedup vs baseline: 1.6021x; 1.0017x over previous
"""Bass/Trainium2 kernel for nn_BuildLstmUnrollNet (bf16+fp8 hybrid).

Problem: 2-layer LSTM, unrolled T=11 steps with per-step (non-shared)
weights, B=8192, R=425, IN=20.  Output block t is the last-layer h
*before* step t, so only steps 0..9 need computing.

Strategy (data-parallel over batch, 8 cores x 1024 rows):
  - Step 0 runs its matmuls in bf16 (the initial h/c are raw unbounded
    randn; fp8-quantizing them costs ~6x the tolerable error).  Steps
    1..9 run all matmuls in fp8e4 with perf_mode=DoubleRow (2 K-rows
    per PE cell, 2x throughput): weights are the moving operand
    (pre-scaled x256, g-gate columns additionally x2), transposed
    activations are stationary, 256 K-features per pass.  Post-step-0
    h's are tanh-bounded, so fp8 keeps absmax rel err ~4x under the
    2e-2 gate.
  - Packed batch-major state (bf16): [h0(425) | 1 | x(20) | pad ->512 |
    h1(425) | pad ->1024].  Bias rides the ones column; layer 0
    contracts features 0..511, layer 1 contracts 0..1023 (x rows
    zero-weighted).  The recurrent transpose bounces through DRAM in
    bf16 (2-byte x-bar DMA transpose) into a [128, 8 chunks, 1024]
    K-major buffer; the otherwise-idle GPSIMD/Pool engine then copies
    it to fp8.  DoubleRow pairs adjacent 128-row chunks (pair stride
    1024, 16-aligned, per the s3_lw dual-fp8 ISA restrictions).
  - One single Sigmoid ACT op per (m-tile, layer) covers ALL 1700 gate
    columns: tanh(g) = 2*sigmoid(2g)-1 with the g columns' weights
    doubled on the host; the affine fix-up is a cheap 4x-mode DVE
    tensor_scalar.  PSUM descale (1/256) rides the ACT scale input.
  - Cell math in fp16 on DVE (2x mode), batched over 4-m-tile groups;
    tanh(c) on ACT batched per group.
  - h1 output is stored bf16 (straight from the packed state) and
    upcast on the host.

kernel(**inputs) takes full-size numpy inputs, does the host-side
packing/sharding, runs the same program SPMD on cores 0..7, and
reassembles the full [8192, 4675] fp32 output (block 0 comes straight
from the initial state on the host).
"""

import numpy as np
import ml_dtypes

FP8 = ml_dtypes.float8_e4m3     # TRN float8e4: max normal 240, inf above
BF16 = ml_dtypes.bfloat16
FP16 = np.float16

B = 8192
NCORES = 8
BC = B // NCORES          # batch rows per core (1024)
NB = BC // 128            # m-tiles per core (8)
R = 425
IN = 20
GN = 4 * R                # 1700 gate columns
GNP = 1712                # padded gate cols in the fp8 weight tile (16|GNP)
SW = 256.0                # fp8 weight scale, descaled via ACT scale=1/SW
ONES_COL = R              # 425: ones feature (bias row rides here)
X_COL = R + 1             # 426..445: x features
H1_OFF = 512              # h1 features at 512..936
SB = 1024                 # packed state width = 8 chunks of 128
NSLOT = 6                 # fp8 weight pair-slots: L0 p0,p1 + L1 p0..p3
NCHUNKS = [(0, 512), (512, 512), (1024, 512), (1536, 164)]
GRPS = [(0, 3), (3, 5)]   # m-tile groups for the cell-math pipeline
NSTEPS = 10

# set by test.py to profile; results stashed in LAST_RESULT
TRACE = False
LAST_RESULT = None


def build_bass(n_steps=NSTEPS, finalize=True):
    import concourse.bacc as bacc
    import concourse.mybir as mybir
    import concourse.tile as tile

    f32 = mybir.dt.float32
    bf16 = mybir.dt.bfloat16
    fp16 = mybir.dt.float16
    f8 = mybir.dt.float8e4
    Sig = mybir.ActivationFunctionType.Sigmoid
    Tanh = mybir.ActivationFunctionType.Tanh
    DR = mybir.MatmulPerfMode.DoubleRow
    mult = mybir.AluOpType.mult
    add = mybir.AluOpType.add

    nc = bacc.Bacc()

    w_d = nc.declare_dram_parameter("w", [n_steps, 128, NSLOT, 2, GNP], f8,
                                    False)
    wb_d = nc.declare_dram_parameter("wb", [128, 12, GN], bf16, False)
    htci_d = nc.declare_dram_parameter("htci", [128, 8, BC], bf16, False)
    hsbi_d = nc.declare_dram_parameter("hsbi", [128, NB, SB], bf16, False)
    c0i_d = nc.declare_dram_parameter("c0i", [128, NB, R], fp16, False)
    c1i_d = nc.declare_dram_parameter("c1i", [128, NB, R], fp16, False)
    out_d = nc.declare_dram_parameter("out", [BC, n_steps * R], bf16, True)
    # DRAM bounce buffer for the recurrent transposes
    hd = nc.dram_tensor("hd", [BC, SB], bf16)

    # fp8 pair-slot (within the per-layer weight tile) -> first state chunk
    L0_CH = (0, 2)            # layer-0 pairs: chunks (0,1), (2,3)
    L1_CH = (0, 2, 4, 6)      # layer-1 pairs: chunks (0,1)..(6,7)

    with tile.TileContext(nc) as tc:
        with (
            tc.tile_pool(name="consts", bufs=1) as consts,
            tc.tile_pool(name="wl0p", bufs=2) as wl0p,
            tc.tile_pool(name="wl1p", bufs=2) as wl1p,
            tc.tile_pool(name="w0pool", bufs=1) as w0pool,
            tc.tile_pool(name="gpsum", bufs=2, space="PSUM") as gpsum,
        ):
            # persistent state tiles
            hsb = consts.tile([128, NB, SB], bf16)      # packed batch-major
            htcb = consts.tile([128, 8, BC], bf16)      # transposed bf16
            # fp8 transposed state, version-rotated so every reader is a
            # full wave behind the writer (h0 side needs 3 live versions,
            # h1 side 2)
            ht03a = consts.tile([128, 4, BC], f8)
            ht03b = consts.tile([128, 4, BC], f8)
            ht03c = consts.tile([128, 4, BC], f8)
            ht47a = consts.tile([128, 4, BC], f8)
            ht47b = consts.tile([128, 4, BC], f8)
            ht03 = [ht03a, ht03b, ht03c]
            ht47 = [ht47a, ht47b]
            c0 = consts.tile([128, NB, R], fp16)
            c1 = consts.tile([128, NB, R], fp16)
            ts0 = consts.tile([128, NB, GN], fp16)      # L0 sigmoid outputs
            ts1 = consts.tile([128, NB, GN], fp16)      # L1 sigmoid outputs
            tg16 = consts.tile([128, NB, R], fp16)
            tc16 = consts.tile([128, NB, R], fp16)

            # step-0 bf16 weights: one tile, L0 chunks then L1 chunks
            wb = w0pool.tile([128, 8, GN], bf16)

            # init DMAs on the SP (HWDGE) queue, most-urgent first; step-0
            # bf16 weights + step-1 fp8 weights on the Pool (SWDGE) queue
            for c in range(4):
                nc.sync.dma_start(htcb[:, c], htci_d[:, c])
            nc.gpsimd.dma_start(wb[:, 0:4], wb_d[:, 0:4])
            nc.sync.dma_start(c0[:], c0i_d[:])
            nc.sync.dma_start(hsb[:], hsbi_d[:])
            for c in range(4, 8):
                nc.sync.dma_start(htcb[:, c], htci_d[:, c])
            nc.sync.dma_start(c1[:], c1i_d[:])
            # L1(0)'s h1-side bf16 weights have no WAR on the L0 half:
            # stream them during the prologue
            for q in range(2):
                nc.gpsimd.dma_start(wb[:, 4 + 2 * q:6 + 2 * q],
                                    wb_d[:, 8 + 2 * q:10 + 2 * q])

            # L1(0)'s h0-side bf16 weights ride a wl1p buffer (same byte
            # budget as an fp8 wl1 tile) so the load has no WAR on wb
            wbl1h0 = wl1p.tile([128, 4, GN], bf16, tag="wl1")
            for q in range(2):
                nc.gpsimd.dma_start(wbl1h0[:, 2 * q:2 * q + 2],
                                    wb_d[:, 4 + 2 * q:6 + 2 * q])
            wl0t = {}   # step -> fp8 L0 weight tile [128, 2, 2, GNP]
            wl1t = {}   # step -> fp8 L1 weight tile [128, 4, 2, GNP]
            if n_steps > 1:
                wl0t[1] = wl0p.tile([128, 2, 2, GNP], f8, tag="wl0", name="wl0_1")
                nc.gpsimd.dma_start(wl0t[1][:], w_d[1][:, 0:2])
            if n_steps > 2:
                wl0t[2] = wl0p.tile([128, 2, 2, GNP], f8, tag="wl0", name="wl0_2")
                nc.gpsimd.dma_start(wl0t[2][:], w_d[2][:, 0:2])

            # PE warm-up: the HAM clock gate needs ~3.4us of sustained
            # activity before the PE runs at full rate.
            warm = consts.tile([128, 128], bf16)
            nc.vector.memset(warm[:], 0.0)
            wps = gpsum.tile([128, 512], f32, tag="g")
            for i in range(20):
                nc.tensor.matmul(wps[:, 0:128], warm[:], warm[:],
                                 start=True, stop=True)

            def mm_sigma(t, layer, m):
                """Gate matmuls + the single whole-width sigmoid for m.
                fp8 stationary versions: layer-0 of step t reads h0(t-1) =
                ht03[(t-1)%3]; layer-1 of step t reads h0(t) = ht03[t%3]
                and h1(t-1) = ht47[t%2]."""
                g = gpsum.tile([128, GN], f32, tag="g")
                if t == 0:
                    chunks = range(0, 4) if layer == 0 else range(0, 8)
                    nch = len(chunks)
                    for ki, ch in enumerate(chunks):
                        lhsT = htcb[:, ch, m * 128:(m + 1) * 128]
                        if layer == 1 and ch < 4:
                            wsrc = wbl1h0[:, ch]
                        else:
                            wsrc = wb[:, ch]
                        for (no, nw) in NCHUNKS:
                            nc.tensor.matmul(
                                g[:, no:no + nw], lhsT,
                                wsrc[:, no:no + nw],
                                start=(ki == 0), stop=(ki == nch - 1))
                else:
                    if layer == 0:
                        srcs = [(ht03[(t - 1) % 3], 0), (ht03[(t - 1) % 3], 2)]
                        wt = wl0t[t]
                    else:
                        h0v, h1v = ht03[t % 3], ht47[t % 2]
                        srcs = [(h0v, 0), (h0v, 2), (h1v, 0), (h1v, 2)]
                        wt = wl1t[t]
                    for si, (ht, ch) in enumerate(srcs):
                        lhsT = ht[:, ch:ch + 2, m * 128:(m + 1) * 128]
                        for (no, nw) in NCHUNKS:
                            nc.tensor.matmul(
                                g[:, no:no + nw], lhsT,
                                wt[:, si, :, no:no + nw],
                                start=(si == 0), stop=(si == len(srcs) - 1),
                                perf_mode=DR)
                # tanh(g) = 2*sigmoid(2g)-1, g-col weights x2 on the host
                tsl = ts0 if layer == 0 else ts1
                nc.scalar.activation(tsl[:, m], g[:], Sig,
                                     scale=(1.0 if t == 0 else 1.0 / SW))

            def cells(t, layer, g0, gl):
                """Batched fp16 cell math + state/output writes for a
                group.  Dead ts gate slots are reused as scratch; the fp8
                stationary convert targets the next wave's parity tile."""
                rows = slice(g0 * 128, (g0 + gl) * 128)
                tsl = ts0 if layer == 0 else ts1
                cst = c0 if layer == 0 else c1
                tcl = tc16
                hoff = 0 if layer == 0 else H1_OFF
                # 2-m-tile sub-batches: the DVE chain for the first pair
                # overlaps the later sigmas, so the bounce fires earlier
                for s0 in range(g0, g0 + gl, 1):
                    ms = slice(s0, min(s0 + 1, g0 + gl))
                    s_i = tsl[:, ms, 0:R]
                    s_f = tsl[:, ms, R:2 * R]
                    s_o = tsl[:, ms, 2 * R:3 * R]
                    s_g = tsl[:, ms, 3 * R:4 * R]
                    nc.vector.tensor_scalar(tg16[:, ms], s_g, 2.0, -1.0,
                                            mult, add)
                    nc.vector.tensor_mul(s_g, s_i, tg16[:, ms])   # i*tanh(g)
                    nc.vector.tensor_mul(s_i, s_f, cst[:, ms])    # f*c
                    nc.vector.tensor_add(cst[:, ms], s_i, s_g)    # c'
                    nc.scalar.activation(tcl[:, ms], cst[:, ms], Tanh)
                    nc.vector.tensor_mul(hsb[:, ms, hoff:hoff + R], s_o,
                                         tcl[:, ms])
                msg = slice(g0, g0 + gl)
                if layer == 1:
                    nc.sync.dma_start(
                        out_d[rows, t * R:(t + 1) * R]
                        .rearrange("(m p) c -> p m c", p=128),
                        hsb[:, msg, H1_OFF:H1_OFF + R])
                if layer == 0 or t < n_steps - 1:
                    # bounce the updated packed half to DRAM, x-bar
                    # transpose back K-major (bf16), then fp8-ify on the
                    # Pool engine for the DoubleRow stationary
                    nc.sync.dma_start(
                        hd[rows, hoff:hoff + 512]
                        .rearrange("(m p) c -> p m c", p=128),
                        hsb[:, msg, hoff:hoff + 512])
                    ch0 = 0 if layer == 0 else 4
                    for ch in range(ch0, ch0 + 4):
                        nc.sync.dma_start(
                            out=htcb[:, ch, rows],
                            in_=hd[rows, 128 * ch:128 * ch + 128],
                            transpose=True)
                    dst = ht03[t % 3] if layer == 0 else ht47[(t + 1) % 2]
                    # two 2-chunk fp8ify ops: downstream DoubleRow pairs can
                    # start after the first pair's chunks land
                    nc.gpsimd.tensor_copy(dst[:, 0:2, rows],
                                          htcb[:, ch0:ch0 + 2, rows])
                    nc.gpsimd.tensor_copy(dst[:, 2:4, rows],
                                          htcb[:, ch0 + 2:ch0 + 4, rows])

            # prologue: L0(0) and L1(0) matmuls in bf16 (L1(0) must read
            # htcb's h0(0) before L0(1)'s transposes overwrite it), then
            # L0(1) in fp8 (its stationary comes from L0(0)'s cells; this
            # chain stalls once, ~10us)
            for (g0, gl) in GRPS:
                for m in range(g0, g0 + gl):
                    mm_sigma(0, 0, m)
                cells(0, 0, g0, gl)

            for (g0, gl) in GRPS:
                for m in range(g0, g0 + gl):
                    mm_sigma(0, 1, m)
            if n_steps > 1:
                for (g0, gl) in GRPS:
                    for m in range(g0, g0 + gl):
                        mm_sigma(1, 0, m)
                    cells(1, 0, g0, gl)

            # waves: L1(t) runs alongside L0(t+1).  All matmuls+sigmas are
            # hoisted to the wave front (they depend only on the previous
            # wave's state), so the recurrent bounce->transpose->fp8ify
            # chain hides under a full wave of ACT work; the fp8 stationary
            # ping-pongs by wave parity to kill cross-wave WAR hazards.
            # waves: wave k = {L0(k+2), L1(k)} — every recurrent
            # dependency (h0 and h1 transposed+fp8ified state, weights) is
            # produced at least one full wave before its consumer, so the
            # bounce->transpose->fp8ify chains hide completely
            for k in range(n_steps - 1):
                for (g0, gl) in GRPS:
                    # L1 leads: it reads two-wave-old h0 state, and its h1
                    # chain is consumed at the very start of wave k+1
                    if k >= 1:
                        # k == 0: L1(0) sigmas already ran in the prologue
                        for m in range(g0, g0 + gl):
                            mm_sigma(k, 1, m)
                    if k + 2 <= n_steps - 1:
                        for m in range(g0, g0 + gl):
                            mm_sigma(k + 2, 0, m)
                    cells(k, 1, g0, gl)
                    if k + 2 <= n_steps - 1:
                        cells(k + 2, 0, g0, gl)
                # weight prefetches at the wave tail: their WAR waits (on
                # the previous tile buffer) must not head-of-line-block the
                # Pool FIFO in front of the fp8ify converts
                wl1t[k + 1] = wl1p.tile([128, 4, 2, GNP], f8, tag="wl1",
                                        name=f"wl1_{k+1}")
                for q in range(2):
                    nc.gpsimd.dma_start(
                        wl1t[k + 1][:, 2 * q:2 * q + 2],
                        w_d[k + 1][:, 2 + 2 * q:4 + 2 * q])
                if k + 3 <= n_steps - 1:
                    wl0t[k + 3] = wl0p.tile([128, 2, 2, GNP], f8, tag="wl0",
                                            name=f"wl0_{k+3}")
                    nc.gpsimd.dma_start(wl0t[k + 3][:], w_d[k + 3][:, 0:2])

            # epilogue: layer 1 of the last step
            for (g0, gl) in GRPS:
                if n_steps > 1:
                    for m in range(g0, g0 + gl):
                        mm_sigma(n_steps - 1, 1, m)
                cells(n_steps - 1, 1, g0, gl)
    if finalize:
        nc.finalize()
    return nc


def prep_inputs(x, init_states_input, W_i2h0, b_i2h0, W_h2h0, b_h2h0,
                W_i2h1, b_i2h1, W_h2h1, b_h2h1, n_steps=NSTEPS):
    """Host-side packing.  Returns (in_maps, h1_init_full)."""
    x = np.asarray(x, np.float32)
    init = np.asarray(init_states_input, np.float32)
    W_i2h0 = np.asarray(W_i2h0, np.float32)[:n_steps]
    b_i2h0 = np.asarray(b_i2h0, np.float32)[:n_steps]
    W_h2h0 = np.asarray(W_h2h0, np.float32)[:n_steps]
    b_h2h0 = np.asarray(b_h2h0, np.float32)[:n_steps]
    W_i2h1 = np.asarray(W_i2h1, np.float32)[:n_steps]
    b_i2h1 = np.asarray(b_i2h1, np.float32)[:n_steps]
    W_h2h1 = np.asarray(W_h2h1, np.float32)[:n_steps]
    b_h2h1 = np.asarray(b_h2h1, np.float32)[:n_steps]

    # per-step K-major weight blocks, rows matching the packed state
    WL0 = np.zeros((n_steps, 512, GN), np.float32)
    WL0[:, 0:R] = W_h2h0.transpose(0, 2, 1)
    WL0[:, ONES_COL] = b_i2h0 + b_h2h0
    WL0[:, X_COL:X_COL + IN] = W_i2h0.transpose(0, 2, 1)
    WL1 = np.zeros((n_steps, SB, GN), np.float32)
    WL1[:, 0:R] = W_i2h1.transpose(0, 2, 1)
    WL1[:, ONES_COL] = b_i2h1 + b_h2h1
    WL1[:, H1_OFF:H1_OFF + R] = W_h2h1.transpose(0, 2, 1)
    for Wx in (WL0, WL1):
        Wx[:, :, 3 * R:] *= 2.0     # g-cols doubled: tanh via sigmoid

    # step-0 bf16 weights: 12 K-chunks of 128 (L0 c0..3, L1 c0..7)
    wb = np.concatenate([WL0[0].reshape(4, 128, GN),
                         WL1[0].reshape(8, 128, GN)], axis=0) \
        .transpose(1, 0, 2)                       # [128, 12, GN]
    wb = np.ascontiguousarray(wb.astype(BF16))

    # fp8 step weights: pair-slot j covers chunks (2j, 2j+1);
    # k = 128*(2j+i) + p  ->  [T, p, slot, i, n], n padded to GNP
    w8f = np.concatenate([
        WL0.reshape(n_steps, 2, 2, 128, GN).transpose(0, 3, 1, 2, 4),
        WL1.reshape(n_steps, 4, 2, 128, GN).transpose(0, 3, 1, 2, 4),
    ], axis=2) * SW                               # [T, 128, 6, 2, GN]
    w8 = np.zeros((n_steps, 128, NSLOT, 2, GNP), FP8)
    w8[..., :GN] = FP8(np.clip(w8f, -240.0, 240.0))

    init4 = init.reshape(B, 4, R)
    h0_full, c0_full = init4[:, 0], init4[:, 1]
    h1_full, c1_full = init4[:, 2], init4[:, 3]

    in_maps = []
    for cidx in range(NCORES):
        sl = slice(cidx * BC, (cidx + 1) * BC)
        hsp = np.zeros((BC, SB), np.float32)
        hsp[:, 0:R] = h0_full[sl]
        hsp[:, ONES_COL] = 1.0
        hsp[:, X_COL:X_COL + IN] = x[sl]
        hsp[:, H1_OFF:H1_OFF + R] = h1_full[sl]
        hspb = hsp.astype(BF16)
        in_maps.append({
            "w": w8,
            "wb": wb,
            "htci": np.ascontiguousarray(
                hspb.reshape(BC, 8, 128).transpose(2, 1, 0)),
            "hsbi": np.ascontiguousarray(
                hspb.reshape(NB, 128, SB).transpose(1, 0, 2)),
            "c0i": np.ascontiguousarray(
                c0_full[sl].astype(FP16).reshape(NB, 128, R)
                .transpose(1, 0, 2)),
            "c1i": np.ascontiguousarray(
                c1_full[sl].astype(FP16).reshape(NB, 128, R)
                .transpose(1, 0, 2)),
        })
    return in_maps, h1_full


def kernel(x, init_states_input, W_i2h0, b_i2h0, W_h2h0, b_h2h0,
           W_i2h1, b_i2h1, W_h2h1, b_h2h1):
    global LAST_RESULT
    from concourse.bass_utils import run_bass_kernel_spmd

    in_maps, h1_full = prep_inputs(
        x, init_states_input, W_i2h0, b_i2h0, W_h2h0, b_h2h0,
        W_i2h1, b_i2h1, W_h2h1, b_h2h1)

    nc = build_bass(NSTEPS)
    res = run_bass_kernel_spmd(nc, in_maps, list(range(NCORES)), trace=TRACE)
    LAST_RESULT = res

    out = np.empty((B, (NSTEPS + 1) * R), np.float32)
    out[:, 0:R] = h1_full
    for c in range(NCORES):
        out[c * BC:(c + 1) * BC, R:] = \
            np.asarray(res.results[c]["out"]).astype(np.float32)
    return out
